# revision 15
# baseline (speedup 1.0000x reference)
"""Dual-state linear attention Trainium2 kernel (8 NeuronCores, SPMD).

Sharding: core = (batch b, head-group g): b = core // 2, g = core % 2.
Each core processes batch b and heads 8g..8g+7 (feature-sharded w_qkv /
w_out slices).  The out-projection partial sums of the two head groups of
each batch are added on the host.

On-chip layout: feature-on-partition, time-on-free ("transposed").
 - qkv matmul:  psum[j 128, t TC] = sum_a wq[128a:+128, j-tile].T @ xT[a][:, chunk]
 - phi(x) = elu(x)+1 = relu(x) + exp(min(x, 0)):
     rneg = Relu(-x) (ACT), e = Exp(-rneg) (ACT),
     phi  = (x max 0) + e   (DVE scalar_tensor_tensor, drains PSUM)
 - decay scans: DVE tensor_tensor_scan along free dim, fp32 decay tiles,
     chained across chunks via initial=prev[:, -1:]
 - den[h,t] = sum_d q*run_ks: PE matmul with block-ones selector [128,2]
 - 1/den: ACT Reciprocal on the 8 written psum rows -> compact [8, TC]
 - broadcast den_inv back to 128 rows: PE matmul with selector [8, 128]
 - Y = q * run_kv * den_inv_bcast (DVE)
 - out[t, o] = sum_y Y.T[y, t].T @ wo[y, o] (PE), drained bf16, DMA out.
"""
import sys

sys.path.insert(0, "/opt/trn_rl_repo")

import numpy as np
import ml_dtypes

import concourse.bacc as bacc
import concourse.tile as tile
from concourse import mybir
from concourse.bass_utils import run_bass_kernel_spmd

BF16 = ml_dtypes.bfloat16

B, T, HID, H, D = 4, 4096, 1024, 16, 64
NCORES = 8
TC = 512            # time-chunk
NG = 4              # feature partition-groups per core (8 heads x 64 = 512 rows)

_BUILD_CACHE = {}


def build(t=T, tc=TC):
    """Build the SPMD Bass program. Returns (nc, names)."""
    chunks = t // tc
    nc = bacc.Bacc("TRN2", target_bir_lowering=False, debug=False,
                   enable_asserts=False, num_devices=NCORES)
    f32, bf16 = mybir.dt.float32, mybir.dt.bfloat16

    xT = nc.dram_tensor("xT", [8, 128, t], bf16, kind="ExternalInput").ap()
    wq = nc.dram_tensor("wq", [8, 128, 1536], bf16, kind="ExternalInput").ap()
    wo = nc.dram_tensor("wo", [8, 128, 1024], bf16, kind="ExternalInput").ap()
    deca = nc.dram_tensor("deca", [NG, 128, tc], f32, kind="ExternalInput").ap()
    decb = nc.dram_tensor("decb", [NG, 128, tc], f32, kind="ExternalInput").ap()
    esel = nc.dram_tensor("esel", [NG, 128, 128], bf16, kind="ExternalInput").ap()
    ehead = nc.dram_tensor("ehead", [NG, 128, 128], bf16, kind="ExternalInput").ap()
    yout = nc.dram_tensor("yout", [t, 1024], bf16, kind="ExternalOutput").ap()
    finals = nc.dram_tensor("finals", [128, 16], bf16, kind="ExternalOutput").ap()

    Relu = mybir.ActivationFunctionType.Relu
    Exp = mybir.ActivationFunctionType.Exp
    Ln = mybir.ActivationFunctionType.Ln
    MUL = mybir.AluOpType.mult
    ADD = mybir.AluOpType.add
    MAX = mybir.AluOpType.max

    with tile.TileContext(nc) as tc_:
        import contextlib
        ctx = contextlib.ExitStack()
        with ctx:
            const = ctx.enter_context(tc_.tile_pool(name="const", bufs=1))
            px = ctx.enter_context(tc_.tile_pool(name="px", bufs=2))
            pact = ctx.enter_context(tc_.tile_pool(name="pact", bufs=3))
            pqk = ctx.enter_context(tc_.tile_pool(name="pqk", bufs=6))
            pscan = ctx.enter_context(tc_.tile_pool(name="pscan", bufs=6))
            pmid = ctx.enter_context(tc_.tile_pool(name="pmid", bufs=6))
            py = ctx.enter_context(tc_.tile_pool(name="py", bufs=6))
            pden = ctx.enter_context(tc_.tile_pool(name="pden", bufs=2))
            pout = ctx.enter_context(tc_.tile_pool(name="pout", bufs=2))
            ps_qkv = ctx.enter_context(
                tc_.tile_pool(name="ps_qkv", bufs=1, space="PSUM"))
            ps_den = ctx.enter_context(
                tc_.tile_pool(name="ps_den", bufs=1, space="PSUM"))
            ps_bc = ctx.enter_context(
                tc_.tile_pool(name="ps_bc", bufs=1, space="PSUM"))
            ps_out = ctx.enter_context(
                tc_.tile_pool(name="ps_out", bufs=2, space="PSUM"))

            # constants / weights resident in SBUF
            wq_sb = const.tile([128, 8, 1536], bf16)
            wo_sb = const.tile([128, 8, 1024], bf16)
            for a in range(8):
                nc.sync.dma_start(out=wq_sb[:, a, :], in_=wq[a])
                nc.sync.dma_start(out=wo_sb[:, a, :], in_=wo[a])
            deca_sb = const.tile([128, NG, tc], f32)
            decb_sb = const.tile([128, NG, tc], f32)
            for j in range(NG):
                nc.sync.dma_start(out=deca_sb[:, j, :], in_=deca[j])
                nc.sync.dma_start(out=decb_sb[:, j, :], in_=decb[j])
            esel_sb = const.tile([128, NG, 128], bf16)
            for j in range(NG):
                nc.sync.dma_start(out=esel_sb[:, j, :], in_=esel[j])
            ehead_sb = const.tile([128, NG, 128], bf16)
            for j in range(NG):
                nc.sync.dma_start(out=ehead_sb[:, j, :], in_=ehead[j])
            epsb = const.tile([128, 1], f32)
            nc.vector.memset(epsb, 1e-20)

            prev = {tag: [None] * NG
                    for tag in ("rkvf", "rkvs", "rksf", "rkss")}

            for c in range(chunks):
                t0 = c * tc
                xc = px.tile([128, 8, tc], bf16, tag="xc")
                for a in range(8):
                    nc.sync.dma_start(out=xc[:, a, :], in_=xT[a, :, t0:t0 + tc])

                tmpf_j, tmps_j = [None] * NG, [None] * NG
                numf_j, nums_j = [None] * NG, [None] * NG
                yf_j, ys_j = [None] * NG, [None] * NG

                for j in range(NG):
                    # --- qkv matmuls for this feature group ---
                    psq = ps_qkv.tile([128, tc], f32, tag="psq")
                    psk = ps_qkv.tile([128, tc], f32, tag="psk")
                    psv = ps_qkv.tile([128, tc], f32, tag="psv")
                    for a in range(8):
                        st, sp = (a == 0), (a == 7)
                        nc.tensor.matmul(
                            psq, wq_sb[:, a, 128 * j:128 * j + 128],
                            xc[:, a, :], start=st, stop=sp)
                        nc.tensor.matmul(
                            psk, wq_sb[:, a, 512 + 128 * j:512 + 128 * j + 128],
                            xc[:, a, :], start=st, stop=sp)
                        nc.tensor.matmul(
                            psv, wq_sb[:, a, 1024 + 128 * j:1024 + 128 * j + 128],
                            xc[:, a, :], start=st, stop=sp)

                    # --- phi(q), phi(k) ---
                    qsb = pqk.tile([128, tc], bf16, tag="q")
                    ksb = pqk.tile([128, tc], bf16, tag="k")
                    for src, dst in ((psq, qsb), (psk, ksb)):
                        rneg = pact.tile([128, tc], f32, tag="rneg")
                        nc.scalar.activation(rneg, src, Relu, scale=-1.0)
                        ex = pact.tile([128, tc], f32, tag="ex")
                        nc.scalar.activation(ex, rneg, Exp, scale=-1.0)
                        nc.vector.scalar_tensor_tensor(
                            dst, src, 0.0, ex, MAX, ADD)

                    # --- kv = phi(k) * v ---
                    kvt = pqk.tile([128, tc], bf16, tag="kv")
                    nc.vector.tensor_mul(kvt, ksb, psv)

                    # --- four decay scans ---
                    souts = {}
                    for tag, src, dsb in (
                            ("rkvf", kvt, deca_sb), ("rkvs", kvt, decb_sb),
                            ("rksf", ksb, deca_sb), ("rkss", ksb, decb_sb)):
                        so = pscan.tile([128, tc], bf16, tag=tag)
                        init = (0.0 if c == 0
                                else prev[tag][j][:, tc - 1:tc])
                        nc.vector.tensor_tensor_scan(
                            so, dsb[:, j, :], src, init, MUL, ADD)
                        prev[tag][j] = so
                        souts[tag] = so

                    # --- tmp = q*run_ks ; num = q*run_kv ---
                    tmpf = pmid.tile([128, tc], bf16, tag="tmpf")
                    nc.vector.tensor_mul(tmpf, qsb, souts["rksf"])
                    tmps = pmid.tile([128, tc], bf16, tag="tmps")
                    nc.vector.tensor_mul(tmps, qsb, souts["rkss"])
                    numf = pmid.tile([128, tc], bf16, tag="numf")
                    nc.vector.tensor_mul(numf, qsb, souts["rkvf"])
                    nums = pmid.tile([128, tc], bf16, tag="nums")
                    nc.vector.tensor_mul(nums, qsb, souts["rkvs"])
                    tmpf_j[j], tmps_j[j] = tmpf, tmps
                    numf_j[j], nums_j[j] = numf, nums

                # --- denominators: selector matmuls accumulated over groups,
                # group j's two head-sums land at rows 32j, 32j+1 ---
                dpf = ps_den.tile([128, tc], f32, tag="dpf")
                dps = ps_den.tile([128, tc], f32, tag="dps")
                for j in range(NG):
                    st, sp = (j == 0), (j == NG - 1)
                    nc.tensor.matmul(dpf, ehead_sb[:, j, :], tmpf_j[j],
                                     start=st, stop=sp)
                    nc.tensor.matmul(dps, ehead_sb[:, j, :], tmps_j[j],
                                     start=st, stop=sp)

                # --- 1/den = Exp(-Ln(den + 1e-20)); unused rows are exact 0,
                # Ln(1e-20) = -46 -> Exp(46) finite, killed by 0 selector ---
                dif = pden.tile([128, tc], bf16, tag="dif")
                dis = pden.tile([128, tc], bf16, tag="dis")
                for dp, di in ((dpf, dif), (dps, dis)):
                    lnd = pact.tile([128, tc], f32, tag="lnd")
                    nc.scalar.activation(lnd, dp, Ln, bias=epsb)
                    nc.scalar.activation(di, lnd, Exp, scale=-1.0)

                # --- broadcast 1/den and Y = num * bcast ---
                for j in range(NG):
                    bcf_ps = ps_bc.tile([128, tc], f32, tag="bc")
                    nc.tensor.matmul(bcf_ps, esel_sb[:, j, :], dif,
                                     start=True, stop=True)
                    bcf = pmid.tile([128, tc], bf16, tag="bcf")
                    nc.scalar.copy(bcf, bcf_ps)
                    yf = py.tile([128, tc], bf16, tag="yf")
                    nc.vector.tensor_mul(yf, numf_j[j], bcf)
                    yf_j[j] = yf

                    bcs_ps = ps_bc.tile([128, tc], f32, tag="bc")
                    nc.tensor.matmul(bcs_ps, esel_sb[:, j, :], dis,
                                     start=True, stop=True)
                    bcs = pmid.tile([128, tc], bf16, tag="bcs")
                    nc.scalar.copy(bcs, bcs_ps)
                    ys = py.tile([128, tc], bf16, tag="ys")
                    nc.vector.tensor_mul(ys, nums_j[j], bcs)
                    ys_j[j] = ys

                # --- out projection ---
                ytiles = yf_j + ys_j
                for ts_ in range(tc // 128):
                    osb = pout.tile([128, 1024], bf16, tag="osb")
                    for ob in range(2):
                        pso = ps_out.tile([128, 512], f32, tag="pso")
                        for yt in range(8):
                            nc.tensor.matmul(
                                pso,
                                ytiles[yt][:, 128 * ts_:128 * ts_ + 128],
                                wo_sb[:, yt, 512 * ob:512 * ob + 512],
                                start=(yt == 0), stop=(yt == 7))
                        nc.scalar.copy(osb[:, 512 * ob:512 * ob + 512], pso)
                    nc.sync.dma_start(
                        out=yout[t0 + 128 * ts_:t0 + 128 * ts_ + 128, :],
                        in_=osb)

            # --- final states: last column of each scan ---
            fin = const.tile([128, 16], bf16)
            for s, tag in enumerate(("rkvf", "rksf", "rkvs", "rkss")):
                for j in range(NG):
                    nc.vector.tensor_copy(fin[:, 4 * s + j:4 * s + j + 1],
                                          prev[tag][j][:, tc - 1:tc])
            nc.sync.dma_start(out=finals, in_=fin)

    nc.compile()
    return nc


def _host_inputs(x, w_qkv, w_out, alpha, beta, t=T, tc=TC):
    """Build the 8 per-core input maps (host-side shard + transpose + cast)."""
    x = np.asarray(x, dtype=np.float32)
    w_qkv = np.asarray(w_qkv, dtype=np.float32)
    w_out = np.asarray(w_out, dtype=np.float32)
    alpha = np.asarray(alpha, dtype=np.float32)
    beta = np.asarray(beta, dtype=np.float32)

    d_a = 1.0 / (1.0 + np.exp(-alpha.astype(np.float64)))
    d_b = 1.0 / (1.0 + np.exp(-beta.astype(np.float64)))
    d_a = d_a.astype(np.float32)
    d_b = d_b.astype(np.float32)

    ehead = np.zeros((NG, 128, 128), dtype=BF16)
    for j in range(NG):
        ehead[j, 0:64, 32 * j] = 1
        ehead[j, 64:128, 32 * j + 1] = 1
    esel = np.zeros((NG, 128, 128), dtype=BF16)
    for j in range(NG):
        esel[j, 32 * j, 0:64] = 1
        esel[j, 32 * j + 1, 64:128] = 1

    in_maps = []
    for core in range(NCORES):
        b, g = core // 2, core % 2
        xt = np.ascontiguousarray(x[b, :t].T).reshape(8, 128, t).astype(BF16)
        qb = w_qkv[:, 512 * g:512 * g + 512]
        kb = w_qkv[:, 1024 + 512 * g:1024 + 512 * g + 512]
        vb = w_qkv[:, 2048 + 512 * g:2048 + 512 * g + 512]
        wqc = np.concatenate([qb, kb, vb], axis=1).reshape(8, 128, 1536)
        wqc = np.ascontiguousarray(wqc).astype(BF16)
        fo = w_out[512 * g:512 * g + 512, :]
        so = w_out[1024 + 512 * g:1024 + 512 * g + 512, :]
        woc = np.concatenate([fo, so], axis=0).reshape(8, 128, 1024)
        woc = np.ascontiguousarray(woc).astype(BF16)

        deca = np.zeros((NG, 128, tc), dtype=np.float32)
        decb = np.zeros((NG, 128, tc), dtype=np.float32)
        for j in range(NG):
            deca[j, 0:64, :] = d_a[8 * g + 2 * j]
            deca[j, 64:128, :] = d_a[8 * g + 2 * j + 1]
            decb[j, 0:64, :] = d_b[8 * g + 2 * j]
            decb[j, 64:128, :] = d_b[8 * g + 2 * j + 1]

        in_maps.append({
            "xT": xt, "wq": wqc, "wo": woc,
            "deca": deca, "decb": decb,
            "esel": esel, "ehead": ehead,
        })
    return in_maps


def _assemble(results, t=T):
    out = np.zeros((B, t, HID), dtype=np.float32)
    kv_f1 = np.zeros((B, H, D), dtype=np.float32)
    ks_f1 = np.zeros((B, H, D), dtype=np.float32)
    kv_s1 = np.zeros((B, H, D), dtype=np.float32)
    ks_s1 = np.zeros((B, H, D), dtype=np.float32)
    for core in range(NCORES):
        b, g = core // 2, core % 2
        out[b] += results[core]["yout"].astype(np.float32)
        fin = results[core]["finals"].astype(np.float32)
        for s, arr in enumerate((kv_f1, ks_f1, kv_s1, ks_s1)):
            for j in range(NG):
                col = fin[:, 4 * s + j]
                arr[b, 8 * g + 2 * j, :] = col[0:64]
                arr[b, 8 * g + 2 * j + 1, :] = col[64:128]
    return out, (kv_f1, ks_f1, kv_s1, ks_s1)


def kernel(x, w_qkv, w_out, alpha, beta, _trace=False):
    key = (T, TC)
    if key not in _BUILD_CACHE:
        _BUILD_CACHE[key] = build(T, TC)
    nc = _BUILD_CACHE[key]
    in_maps = _host_inputs(x, w_qkv, w_out, alpha, beta, T, TC)
    kw = {}
    if _trace:
        kw["trace"] = True
    res = run_bass_kernel_spmd(nc, in_maps, list(range(NCORES)), **kw)
    outs = _assemble(res.results, T)
    if _trace:
        kernel.last_exec_time_ns = res.exec_time_ns
        kernel.last_result = res
    return outs


# revision 39
# speedup vs baseline: 1.2665x; 1.2665x over previous
"""Dual-state linear attention Trainium2 kernel (8 NeuronCores, SPMD).

Sharding: core = (batch b, head-group g): b = core // 2, g = core % 2.
Each core processes batch b and heads 8g..8g+7 (feature-sharded w_qkv /
w_out slices).  The out-projection partial sums of the two head groups of
each batch are added on the host.

On-chip layout: feature-on-partition, time-on-free ("transposed").
 - qkv matmul:  psum[j 128, t TC] = sum_a wq[128a:+128, j-tile].T @ xT[a][:, chunk]
 - phi(x) = elu(x)+1 = relu(x) + exp(min(x, 0)):
     rneg = Relu(-x) (ACT), e = Exp(-rneg) (ACT),
     phi  = (x max 0) + e   (DVE scalar_tensor_tensor, drains PSUM)
 - decay scans: DVE tensor_tensor_scan along free dim, fp32 decay tiles,
     chained across chunks via initial=prev[:, -1:]
 - den[h,t] = sum_d q*run_ks: PE matmul with block-ones selector [128,2]
 - 1/den: ACT Reciprocal on the 8 written psum rows -> compact [8, TC]
 - broadcast den_inv back to 128 rows: PE matmul with selector [8, 128]
 - Y = q * run_kv * den_inv_bcast (DVE)
 - out[t, o] = sum_y Y.T[y, t].T @ wo[y, o] (PE), drained bf16, DMA out.
"""
import sys

sys.path.insert(0, "/opt/trn_rl_repo")

import numpy as np
import ml_dtypes

import concourse.bass as bass
import concourse.bacc as bacc
import concourse.tile as tile
from concourse import mybir
from concourse.bass_utils import run_bass_kernel_spmd

BF16 = ml_dtypes.bfloat16

B, T, HID, H, D = 4, 4096, 1024, 16, 64
NCORES = 8
TC = 512            # time-chunk
NG = 4              # feature partition-groups per core (8 heads x 64 = 512 rows)

_BUILD_CACHE = {}


def build(t=T, tc=TC):
    """Build the SPMD Bass program. Returns (nc, names)."""
    chunks = t // tc
    nc = bacc.Bacc("TRN2", target_bir_lowering=False, debug=False,
                   enable_asserts=False, num_devices=NCORES)
    f32, bf16 = mybir.dt.float32, mybir.dt.bfloat16

    xT = nc.dram_tensor("xT", [8, 128, t], bf16, kind="ExternalInput").ap()
    wq = nc.dram_tensor("wq", [8, 128, 1536], bf16, kind="ExternalInput").ap()
    wo = nc.dram_tensor("wo", [8, 128, 1024], bf16, kind="ExternalInput").ap()
    deca = nc.dram_tensor("deca", [NG, 128, tc], f32, kind="ExternalInput").ap()
    decb = nc.dram_tensor("decb", [NG, 128, tc], f32, kind="ExternalInput").ap()
    esel = nc.dram_tensor("esel", [2 * NG, 128, 128], bf16, kind="ExternalInput").ap()
    ehead = nc.dram_tensor("ehead", [2 * NG, 128, 128], bf16, kind="ExternalInput").ap()
    yout = nc.dram_tensor("yout", [t, 1024], bf16, kind="ExternalOutput").ap()
    finals = nc.dram_tensor("finals", [128, 16], bf16, kind="ExternalOutput").ap()

    Relu = mybir.ActivationFunctionType.Relu
    Exp = mybir.ActivationFunctionType.Exp
    Ln = mybir.ActivationFunctionType.Ln
    MUL = mybir.AluOpType.mult
    ADD = mybir.AluOpType.add
    MAX = mybir.AluOpType.max

    with tile.TileContext(nc) as tc_:
        import contextlib
        ctx = contextlib.ExitStack()
        with ctx:
            const = ctx.enter_context(tc_.tile_pool(name="const", bufs=1))
            px = ctx.enter_context(tc_.tile_pool(name="px", bufs=2))
            pact = ctx.enter_context(tc_.tile_pool(name="pact", bufs=2))
            pqk = ctx.enter_context(tc_.tile_pool(name="pqk", bufs=5))
            pscan = ctx.enter_context(tc_.tile_pool(name="pscan", bufs=8))
            pmid = ctx.enter_context(tc_.tile_pool(name="pmid", bufs=8))
            pbc = ctx.enter_context(tc_.tile_pool(name="pbc", bufs=3))
            py = ctx.enter_context(tc_.tile_pool(name="py", bufs=12))
            pden = ctx.enter_context(tc_.tile_pool(name="pden", bufs=2))
            pout = ctx.enter_context(tc_.tile_pool(name="pout", bufs=2))
            ps_q = ctx.enter_context(
                tc_.tile_pool(name="ps_q", bufs=2, space="PSUM"))
            ps_k = ctx.enter_context(
                tc_.tile_pool(name="ps_k", bufs=2, space="PSUM"))
            ps_v = ctx.enter_context(
                tc_.tile_pool(name="ps_v", bufs=1, space="PSUM"))
            ps_bc = ctx.enter_context(
                tc_.tile_pool(name="ps_bc", bufs=2, space="PSUM"))
            ps_out = ctx.enter_context(
                tc_.tile_pool(name="ps_out", bufs=1, space="PSUM"))

            # constants / weights resident in SBUF (emitted inside the
            # pipeline driver: xc(0) and group-0 blocks first)
            wq_sb = const.tile([128, 8, 12, 128], bf16)
            wqr = wq.rearrange("a p (blk n) -> a p blk n", n=128)

            def load_wq(j):
                for a in range(8):
                    nc.sync.dma_start(out=wq_sb[:, a, 3 * j:3 * j + 3, :],
                                      in_=wqr[a, :, 3 * j:3 * j + 3, :])
            deca_sb = const.tile([128, NG, tc], f32)
            decb_sb = const.tile([128, NG, tc], f32)

            def load_dec():
                for j in range(NG):
                    nc.sync.dma_start(out=deca_sb[:, j, :], in_=deca[j])
                    nc.sync.dma_start(out=decb_sb[:, j, :], in_=decb[j])
            ehead_sb = const.tile([128, 2 * NG, 128], bf16)
            esel_sb = const.tile([128, 2 * NG, 128], bf16)
            wo_sb = const.tile([128, 8, 1024], bf16)
            epsb = const.tile([128, 1], f32)
            nc.vector.memset(epsb, 1e-20)

            def load_late_consts():
                for j in range(2 * NG):
                    nc.sync.dma_start(out=ehead_sb[:, j, :], in_=ehead[j])
                    nc.sync.dma_start(out=esel_sb[:, j, :], in_=esel[j])
                for a in range(8):
                    nc.sync.dma_start(out=wo_sb[:, a, :], in_=wo[a])

            prev = {tag: [None] * NG
                    for tag in ("rkvf", "rkvs", "rksf", "rkss")}
            stash = {}   # chunk -> (tmpf_j, tmps_j, numf_j, nums_j)

            def load_xc(c):
                t0 = c * tc
                xc = px.tile([128, 8, tc], bf16, tag="xc")
                for a in range(8):
                    nc.sync.dma_start(out=xc[:, a, :], in_=xT[a, :, t0:t0 + tc])
                return xc

            def front_group(c, j, xc):
                """qkv matmuls + phi + kv + scans + tmp/num for (chunk, grp)."""
                if j == 0:
                    stash[c] = ([None] * NG, [None] * NG,
                                [None] * NG, [None] * NG)
                tmpf_j, tmps_j, numf_j, nums_j = stash[c]
                if True:
                    psq = ps_q.tile([128, tc], f32, tag="psq")
                    psk = ps_k.tile([128, tc], f32, tag="psk")
                    psv = ps_v.tile([128, tc], f32, tag="psv")
                    for qi, ps in enumerate((psq, psk, psv)):
                        for a in range(8):
                            nc.tensor.matmul(
                                ps, wq_sb[:, a, 3 * j + qi, :],
                                xc[:, a, :], start=(a == 0), stop=(a == 7))

                    qsb = pqk.tile([128, tc], bf16, tag="q")
                    ksb = pqk.tile([128, tc], bf16, tag="k")
                    for src, dst in ((psq, qsb), (psk, ksb)):
                        rneg = pact.tile([128, tc], f32, tag="rneg")
                        nc.scalar.activation(rneg, src, Relu, scale=-1.0)
                        ex = pact.tile([128, tc], f32, tag="ex")
                        nc.scalar.activation(ex, rneg, Exp, scale=-1.0)
                        nc.vector.scalar_tensor_tensor(
                            dst, src, 0.0, ex, MAX, ADD)

                    kvt = pqk.tile([128, tc], bf16, tag="kv")
                    nc.vector.tensor_mul(kvt, ksb, psv)

                    souts = {}
                    for tag, src, dsb in (
                            ("rkvf", kvt, deca_sb), ("rkvs", kvt, decb_sb),
                            ("rksf", ksb, deca_sb), ("rkss", ksb, decb_sb)):
                        so = pscan.tile([128, tc], bf16, tag=tag)
                        init = (0.0 if c == 0
                                else prev[tag][j][:, tc - 1:tc])
                        nc.vector.tensor_tensor_scan(
                            so, dsb[:, j, :], src, init, MUL, ADD)
                        prev[tag][j] = so
                        souts[tag] = so

                    tmpf = pmid.tile([128, tc], bf16, tag="tmpf")
                    nc.vector.tensor_mul(tmpf, qsb, souts["rksf"])
                    tmps = pmid.tile([128, tc], bf16, tag="tmps")
                    nc.vector.tensor_mul(tmps, qsb, souts["rkss"])
                    numf = pmid.tile([128, tc], bf16, tag="numf")
                    nc.vector.tensor_mul(numf, qsb, souts["rkvf"])
                    nums = pmid.tile([128, tc], bf16, tag="nums")
                    nc.vector.tensor_mul(nums, qsb, souts["rkvs"])
                    tmpf_j[j], tmps_j[j] = tmpf, tmps
                    numf_j[j], nums_j[j] = numf, nums

            def stage_tail_a(c):
                """den + 1/den + broadcast + Y for chunk c."""
                tmpf_j, tmps_j, numf_j, nums_j = stash.pop(c)
                yf_j, ys_j = [None] * NG, [None] * NG

                # both decays' denominators in ONE psum bank:
                # group j / decay s / head-parity e at row 32j + 2s + e
                dp = ps_bc.tile([128, tc], f32, tag="bc")
                for i, tm in enumerate(tmpf_j + tmps_j):
                    s, j = divmod(i, NG)
                    nc.tensor.matmul(dp, ehead_sb[:, 4 * s + j, :], tm,
                                     start=(i == 0), stop=(i == 2 * NG - 1))

                # 1/den = Exp(-Ln(den + 1e-20)); unused rows are exact 0,
                # Ln(1e-20) = -46 -> Exp(46) finite, killed by 0 selector.
                dinv = pden.tile([128, tc], bf16, tag="dinv")
                lnd = pact.tile([128, tc], f32, tag="lnd")
                nc.scalar.activation(lnd, dp, Ln, bias=epsb)
                nc.scalar.activation(dinv, lnd, Exp, scale=-1.0)

                def bcast_pe(s, j, tag):
                    """den_inv rows {32j+2s, 32j+2s+1} -> [128, tc] via
                    selector matmul + ACT drain."""
                    bc_ps = ps_bc.tile([128, tc], f32, tag="bc")
                    nc.tensor.matmul(bc_ps, esel_sb[:, 4 * s + j, :], dinv,
                                     start=True, stop=True)
                    bc = pbc.tile([128, tc], bf16, tag=tag)
                    nc.scalar.copy(bc, bc_ps)
                    return bc

                for j in range(NG):
                    bcf = bcast_pe(0, j, "bcf")
                    yf = py.tile([128, tc], bf16, tag="yf")
                    nc.vector.tensor_mul(yf, numf_j[j], bcf)
                    yf_j[j] = yf

                    bcs = bcast_pe(1, j, "bcs")
                    ys = py.tile([128, tc], bf16, tag="ys")
                    nc.vector.tensor_mul(ys, nums_j[j], bcs)
                    ys_j[j] = ys

                ystash[c] = yf_j + ys_j

            def tail_b_slice(c, ts_):
                """out-projection for t-subtile ts_ of chunk c."""
                t0 = c * tc
                ytiles = ystash[c]
                osb = pout.tile([128, 1024], bf16, tag="osb")
                for ob in range(2):
                    pso = ps_out.tile([128, 512], f32, tag="pso")
                    for yt in range(8):
                        nc.tensor.matmul(
                            pso,
                            ytiles[yt][:, 128 * ts_:128 * ts_ + 128],
                            wo_sb[:, yt, 512 * ob:512 * ob + 512],
                            start=(yt == 0), stop=(yt == 7))
                    nc.scalar.copy(osb[:, 512 * ob:512 * ob + 512], pso)
                nc.sync.dma_start(
                    out=yout[t0 + 128 * ts_:t0 + 128 * ts_ + 128, :],
                    in_=osb)
                if ts_ == tc // 128 - 1:
                    del ystash[c]

            ystash = {}
            # 2-deep software pipeline, group-interleaved:
            #   front(c) groups  ||  outproj slices of chunk c-2  ||  tailA(c-1)
            nslice = tc // 128
            load_wq(0)
            xcs = {0: load_xc(0)}
            load_dec()
            for j in range(1, NG):
                load_wq(j)
            for c in range(chunks):
                xc = xcs.pop(c)
                for j in range(NG):
                    front_group(c, j, xc)
                    if c >= 2:
                        for k in range(nslice * j // NG, nslice * (j + 1) // NG):
                            tail_b_slice(c - 2, k)
                if c == 0:
                    load_late_consts()
                if c + 1 < chunks:
                    xcs[c + 1] = load_xc(c + 1)
                if c >= 1:
                    stage_tail_a(c - 1)
            stage_tail_a(chunks - 1)
            for cc in (chunks - 2, chunks - 1):
                if cc >= 0 and cc in ystash:
                    for k in range(nslice):
                        tail_b_slice(cc, k)

            # --- final states: last column of each scan ---
            fin = const.tile([128, 16], bf16)
            for s, tag in enumerate(("rkvf", "rksf", "rkvs", "rkss")):
                for j in range(NG):
                    nc.vector.tensor_copy(fin[:, 4 * s + j:4 * s + j + 1],
                                          prev[tag][j][:, tc - 1:tc])
            nc.sync.dma_start(out=finals, in_=fin)

    nc.compile()
    return nc


def _host_inputs(x, w_qkv, w_out, alpha, beta, t=T, tc=TC):
    """Build the 8 per-core input maps (host-side shard + transpose + cast)."""
    x = np.asarray(x, dtype=np.float32)
    w_qkv = np.asarray(w_qkv, dtype=np.float32)
    w_out = np.asarray(w_out, dtype=np.float32)
    alpha = np.asarray(alpha, dtype=np.float32)
    beta = np.asarray(beta, dtype=np.float32)

    d_a = 1.0 / (1.0 + np.exp(-alpha.astype(np.float64)))
    d_b = 1.0 / (1.0 + np.exp(-beta.astype(np.float64)))
    d_a = d_a.astype(np.float32)
    d_b = d_b.astype(np.float32)

    esel = np.zeros((2 * NG, 128, 128), dtype=BF16)
    ehead = np.zeros((2 * NG, 128, 128), dtype=BF16)
    for s in range(2):
        for j in range(NG):
            r = 32 * j + 2 * s
            esel[4 * s + j, r, 0:64] = 1
            esel[4 * s + j, r + 1, 64:128] = 1
            ehead[4 * s + j, 0:64, r] = 1
            ehead[4 * s + j, 64:128, r + 1] = 1

    in_maps = []
    for core in range(NCORES):
        b, g = core // 2, core % 2
        xt = np.ascontiguousarray(x[b, :t].T).reshape(8, 128, t).astype(BF16)
        blocks = []
        for j in range(NG):
            for off in (0, 1024, 2048):
                c0 = off + 512 * g + 128 * j
                blocks.append(w_qkv[:, c0:c0 + 128])
        wqc = np.concatenate(blocks, axis=1).reshape(8, 128, 1536)
        wqc = np.ascontiguousarray(wqc).astype(BF16)
        fo = w_out[512 * g:512 * g + 512, :]
        so = w_out[1024 + 512 * g:1024 + 512 * g + 512, :]
        woc = np.concatenate([fo, so], axis=0).reshape(8, 128, 1024)
        woc = np.ascontiguousarray(woc).astype(BF16)

        deca = np.zeros((NG, 128, tc), dtype=np.float32)
        decb = np.zeros((NG, 128, tc), dtype=np.float32)
        for j in range(NG):
            deca[j, 0:64, :] = d_a[8 * g + 2 * j]
            deca[j, 64:128, :] = d_a[8 * g + 2 * j + 1]
            decb[j, 0:64, :] = d_b[8 * g + 2 * j]
            decb[j, 64:128, :] = d_b[8 * g + 2 * j + 1]

        in_maps.append({
            "xT": xt, "wq": wqc, "wo": woc,
            "deca": deca, "decb": decb,
            "esel": esel, "ehead": ehead,
        })
    return in_maps


def _assemble(results, t=T):
    out = np.zeros((B, t, HID), dtype=np.float32)
    kv_f1 = np.zeros((B, H, D), dtype=np.float32)
    ks_f1 = np.zeros((B, H, D), dtype=np.float32)
    kv_s1 = np.zeros((B, H, D), dtype=np.float32)
    ks_s1 = np.zeros((B, H, D), dtype=np.float32)
    for core in range(NCORES):
        b, g = core // 2, core % 2
        out[b] += results[core]["yout"].astype(np.float32)
        fin = results[core]["finals"].astype(np.float32)
        for s, arr in enumerate((kv_f1, ks_f1, kv_s1, ks_s1)):
            for j in range(NG):
                col = fin[:, 4 * s + j]
                arr[b, 8 * g + 2 * j, :] = col[0:64]
                arr[b, 8 * g + 2 * j + 1, :] = col[64:128]
    return out, (kv_f1, ks_f1, kv_s1, ks_s1)


def kernel(x, w_qkv, w_out, alpha, beta, _trace=False):
    key = (T, TC)
    if key not in _BUILD_CACHE:
        _BUILD_CACHE[key] = build(T, TC)
    nc = _BUILD_CACHE[key]
    in_maps = _host_inputs(x, w_qkv, w_out, alpha, beta, T, TC)
    kw = {}
    if _trace:
        kw["trace"] = True
    res = run_bass_kernel_spmd(nc, in_maps, list(range(NCORES)), **kw)
    outs = _assemble(res.results, T)
    if _trace:
        kernel.last_exec_time_ns = res.exec_time_ns
        kernel.last_result = res
    return outs


# revision 42
# speedup vs baseline: 1.2780x; 1.0091x over previous
"""Dual-state linear attention Trainium2 kernel (8 NeuronCores, SPMD).

Sharding: core = (batch b, head-group g): b = core // 2, g = core % 2.
Each core processes batch b and heads 8g..8g+7 (feature-sharded w_qkv /
w_out slices).  The out-projection partial sums of the two head groups of
each batch are added on the host.

On-chip layout: feature-on-partition, time-on-free ("transposed"); x is
pre-transposed and bf16-cast on the host so no on-device transposes exist.
 - qkv matmul: psum[j 128, t TC] = sum_a wq[k-tile, j-tile].T @ xT[k-tile, chunk]
 - phi(x) = elu(x)+1 = relu(x) + exp(min(x, 0)):
     rneg = Relu(-x) (ACT), e = Exp(-rneg) (ACT),
     phi  = (x max 0) + e   (DVE scalar_tensor_tensor, drains PSUM)
 - decay scans: DVE tensor_tensor_scan along the free (time) dim, fp32
     decay tiles (bf16 decay would distort 1/(1-d) by ~6%), chained
     across chunks via initial = prev[:, -1:]
 - den[h,t] = sum_d q*run_ks: selector matmuls accumulate both decays'
     head-sums into ONE psum bank at rows 32j + 2s + e
 - 1/den = Exp(-Ln(den + 1e-20)) on ACT (Reciprocal is blocked; unused
     rows are exact 0 -> finite garbage killed by 0 selector weights)
 - broadcast 1/den rows to 64-row blocks: selector matmul + ACT drain
 - Y = (q * run_kv) * den_inv_bcast (DVE, bf16 2x mode)
 - out[t, o] = sum_y Y[y-tile, t-sub].T @ wo[y-tile, o-bank] (PE),
     ACT-drained to bf16, DMA out; host sums the 2 head-group partials.

Scheduling: 2-deep software pipeline, interleaved at feature-group
granularity so the in-order PE queue can fill qkv stalls with the
out-projection of chunk c-2 (whose Y tiles are certainly ready):
  front(c) group j  ||  outproj slice j of chunk c-2, then tailA(c-1).
Measured ~404 us on silicon (PE busy ~331 us, DVE ~291 us of which the
four scans are ~151 us at the hardware's 2.09 cycles/element).
"""
import sys

sys.path.insert(0, "/opt/trn_rl_repo")

import numpy as np
import ml_dtypes

import concourse.bass as bass
import concourse.bacc as bacc
import concourse.tile as tile
from concourse import mybir
from concourse.bass_utils import run_bass_kernel_spmd

BF16 = ml_dtypes.bfloat16

B, T, HID, H, D = 4, 4096, 1024, 16, 64
NCORES = 8
TC = 512            # time-chunk
NG = 4              # feature partition-groups per core (8 heads x 64 = 512 rows)

_BUILD_CACHE = {}


def build(t=T, tc=TC):
    """Build the SPMD Bass program. Returns (nc, names)."""
    chunks = t // tc
    nc = bacc.Bacc("TRN2", target_bir_lowering=False, debug=False,
                   enable_asserts=False, num_devices=NCORES)
    f32, bf16 = mybir.dt.float32, mybir.dt.bfloat16

    xT = nc.dram_tensor("xT", [8, 128, t], bf16, kind="ExternalInput").ap()
    wq = nc.dram_tensor("wq", [8, 128, 1536], bf16, kind="ExternalInput").ap()
    wo = nc.dram_tensor("wo", [8, 128, 1024], bf16, kind="ExternalInput").ap()
    deca = nc.dram_tensor("deca", [NG, 128, tc], f32, kind="ExternalInput").ap()
    decb = nc.dram_tensor("decb", [NG, 128, tc], f32, kind="ExternalInput").ap()
    esel = nc.dram_tensor("esel", [2 * NG, 128, 128], bf16, kind="ExternalInput").ap()
    ehead = nc.dram_tensor("ehead", [2 * NG, 128, 128], bf16, kind="ExternalInput").ap()
    yout = nc.dram_tensor("yout", [t, 1024], bf16, kind="ExternalOutput").ap()
    finals = nc.dram_tensor("finals", [128, 16], bf16, kind="ExternalOutput").ap()

    Relu = mybir.ActivationFunctionType.Relu
    Exp = mybir.ActivationFunctionType.Exp
    Ln = mybir.ActivationFunctionType.Ln
    MUL = mybir.AluOpType.mult
    ADD = mybir.AluOpType.add
    MAX = mybir.AluOpType.max

    with tile.TileContext(nc) as tc_:
        import contextlib
        ctx = contextlib.ExitStack()
        with ctx:
            const = ctx.enter_context(tc_.tile_pool(name="const", bufs=1))
            px = ctx.enter_context(tc_.tile_pool(name="px", bufs=2))
            pact = ctx.enter_context(tc_.tile_pool(name="pact", bufs=2))
            pqk = ctx.enter_context(tc_.tile_pool(name="pqk", bufs=5))
            pscan = ctx.enter_context(tc_.tile_pool(name="pscan", bufs=8))
            pmid = ctx.enter_context(tc_.tile_pool(name="pmid", bufs=8))
            pbc = ctx.enter_context(tc_.tile_pool(name="pbc", bufs=3))
            py = ctx.enter_context(tc_.tile_pool(name="py", bufs=12))
            pden = ctx.enter_context(tc_.tile_pool(name="pden", bufs=2))
            pout = ctx.enter_context(tc_.tile_pool(name="pout", bufs=2))
            ps_q = ctx.enter_context(
                tc_.tile_pool(name="ps_q", bufs=2, space="PSUM"))
            ps_k = ctx.enter_context(
                tc_.tile_pool(name="ps_k", bufs=2, space="PSUM"))
            ps_v = ctx.enter_context(
                tc_.tile_pool(name="ps_v", bufs=1, space="PSUM"))
            ps_bc = ctx.enter_context(
                tc_.tile_pool(name="ps_bc", bufs=2, space="PSUM"))
            ps_out = ctx.enter_context(
                tc_.tile_pool(name="ps_out", bufs=1, space="PSUM"))

            # constants / weights resident in SBUF (emitted inside the
            # pipeline driver: xc(0) and group-0 blocks first)
            wq_sb = const.tile([128, 8, 12, 128], bf16)
            wqr = wq.rearrange("a p (blk n) -> a p blk n", n=128)

            def load_wq(j):
                for a in range(8):
                    nc.sync.dma_start(out=wq_sb[:, a, 3 * j:3 * j + 3, :],
                                      in_=wqr[a, :, 3 * j:3 * j + 3, :])
            deca_sb = const.tile([128, NG, tc], f32)
            decb_sb = const.tile([128, NG, tc], f32)

            def load_dec():
                for j in range(NG):
                    nc.sync.dma_start(out=deca_sb[:, j, :], in_=deca[j])
                    nc.sync.dma_start(out=decb_sb[:, j, :], in_=decb[j])
            ehead_sb = const.tile([128, 2 * NG, 128], bf16)
            esel_sb = const.tile([128, 2 * NG, 128], bf16)
            wo_sb = const.tile([128, 8, 1024], bf16)
            epsb = const.tile([128, 1], f32)
            nc.vector.memset(epsb, 1e-20)

            def load_late_consts():
                for j in range(2 * NG):
                    nc.sync.dma_start(out=ehead_sb[:, j, :], in_=ehead[j])
                    nc.sync.dma_start(out=esel_sb[:, j, :], in_=esel[j])
                for a in range(8):
                    nc.sync.dma_start(out=wo_sb[:, a, :], in_=wo[a])

            prev = {tag: [None] * NG
                    for tag in ("rkvf", "rkvs", "rksf", "rkss")}
            stash = {}   # chunk -> (tmpf_j, tmps_j, numf_j, nums_j)

            def load_xc(c):
                t0 = c * tc
                xc = px.tile([128, 8, tc], bf16, tag="xc")
                for a in range(8):
                    nc.sync.dma_start(out=xc[:, a, :], in_=xT[a, :, t0:t0 + tc])
                return xc

            def front_group(c, j, xc):
                """qkv matmuls + phi + kv + scans + tmp/num for (chunk, grp)."""
                if j == 0:
                    stash[c] = ([None] * NG, [None] * NG,
                                [None] * NG, [None] * NG)
                tmpf_j, tmps_j, numf_j, nums_j = stash[c]
                if True:
                    psq = ps_q.tile([128, tc], f32, tag="psq")
                    psk = ps_k.tile([128, tc], f32, tag="psk")
                    psv = ps_v.tile([128, tc], f32, tag="psv")
                    for qi, ps in enumerate((psq, psk, psv)):
                        for a in range(8):
                            nc.tensor.matmul(
                                ps, wq_sb[:, a, 3 * j + qi, :],
                                xc[:, a, :], start=(a == 0), stop=(a == 7))

                    qsb = pqk.tile([128, tc], bf16, tag="q")
                    ksb = pqk.tile([128, tc], bf16, tag="k")
                    for src, dst in ((psq, qsb), (psk, ksb)):
                        rneg = pact.tile([128, tc], f32, tag="rneg")
                        nc.scalar.activation(rneg, src, Relu, scale=-1.0)
                        ex = pact.tile([128, tc], f32, tag="ex")
                        nc.scalar.activation(ex, rneg, Exp, scale=-1.0)
                        nc.vector.scalar_tensor_tensor(
                            dst, src, 0.0, ex, MAX, ADD)

                    kvt = pqk.tile([128, tc], bf16, tag="kv")
                    nc.vector.tensor_mul(kvt, ksb, psv)

                    souts = {}
                    for tag, src, dsb in (
                            ("rkvf", kvt, deca_sb), ("rkvs", kvt, decb_sb),
                            ("rksf", ksb, deca_sb), ("rkss", ksb, decb_sb)):
                        so = pscan.tile([128, tc], bf16, tag=tag)
                        init = (0.0 if c == 0
                                else prev[tag][j][:, tc - 1:tc])
                        nc.vector.tensor_tensor_scan(
                            so, dsb[:, j, :], src, init, MUL, ADD)
                        prev[tag][j] = so
                        souts[tag] = so

                    tmpf = pmid.tile([128, tc], bf16, tag="tmpf")
                    nc.vector.tensor_mul(tmpf, qsb, souts["rksf"])
                    tmps = pmid.tile([128, tc], bf16, tag="tmps")
                    nc.vector.tensor_mul(tmps, qsb, souts["rkss"])
                    numf = pmid.tile([128, tc], bf16, tag="numf")
                    nc.vector.tensor_mul(numf, qsb, souts["rkvf"])
                    nums = pmid.tile([128, tc], bf16, tag="nums")
                    nc.vector.tensor_mul(nums, qsb, souts["rkvs"])
                    tmpf_j[j], tmps_j[j] = tmpf, tmps
                    numf_j[j], nums_j[j] = numf, nums

            def stage_tail_a(c):
                """den + 1/den + broadcast + Y for chunk c."""
                tmpf_j, tmps_j, numf_j, nums_j = stash.pop(c)
                yf_j, ys_j = [None] * NG, [None] * NG

                # both decays' denominators in ONE psum bank:
                # group j / decay s / head-parity e at row 32j + 2s + e
                dp = ps_bc.tile([128, tc], f32, tag="bc")
                for i, tm in enumerate(tmpf_j + tmps_j):
                    s, j = divmod(i, NG)
                    nc.tensor.matmul(dp, ehead_sb[:, 4 * s + j, :], tm,
                                     start=(i == 0), stop=(i == 2 * NG - 1))

                # 1/den = Exp(-Ln(den + 1e-20)); unused rows are exact 0,
                # Ln(1e-20) = -46 -> Exp(46) finite, killed by 0 selector.
                dinv = pden.tile([128, tc], bf16, tag="dinv")
                lnd = pact.tile([128, tc], f32, tag="lnd")
                nc.scalar.activation(lnd, dp, Ln, bias=epsb)
                nc.scalar.activation(dinv, lnd, Exp, scale=-1.0)

                def bcast_pe(s, j, tag):
                    """den_inv rows {32j+2s, 32j+2s+1} -> [128, tc] via
                    selector matmul + ACT drain."""
                    bc_ps = ps_bc.tile([128, tc], f32, tag="bc")
                    nc.tensor.matmul(bc_ps, esel_sb[:, 4 * s + j, :], dinv,
                                     start=True, stop=True)
                    bc = pbc.tile([128, tc], bf16, tag=tag)
                    nc.scalar.copy(bc, bc_ps)
                    return bc

                for j in range(NG):
                    bcf = bcast_pe(0, j, "bcf")
                    yf = py.tile([128, tc], bf16, tag="yf")
                    nc.vector.tensor_mul(yf, numf_j[j], bcf)
                    yf_j[j] = yf

                    bcs = bcast_pe(1, j, "bcs")
                    ys = py.tile([128, tc], bf16, tag="ys")
                    nc.vector.tensor_mul(ys, nums_j[j], bcs)
                    ys_j[j] = ys

                ystash[c] = yf_j + ys_j

            def tail_b_slice(c, ts_):
                """out-projection for t-subtile ts_ of chunk c."""
                t0 = c * tc
                ytiles = ystash[c]
                osb = pout.tile([128, 1024], bf16, tag="osb")
                for ob in range(2):
                    pso = ps_out.tile([128, 512], f32, tag="pso")
                    for yt in range(8):
                        nc.tensor.matmul(
                            pso,
                            ytiles[yt][:, 128 * ts_:128 * ts_ + 128],
                            wo_sb[:, yt, 512 * ob:512 * ob + 512],
                            start=(yt == 0), stop=(yt == 7))
                    nc.scalar.copy(osb[:, 512 * ob:512 * ob + 512], pso)
                nc.sync.dma_start(
                    out=yout[t0 + 128 * ts_:t0 + 128 * ts_ + 128, :],
                    in_=osb)
                if ts_ == tc // 128 - 1:
                    del ystash[c]

            ystash = {}
            # 2-deep software pipeline, group-interleaved:
            #   front(c) groups  ||  outproj slices of chunk c-2  ||  tailA(c-1)
            nslice = tc // 128
            load_wq(0)
            xcs = {0: load_xc(0)}
            load_dec()
            for j in range(1, NG):
                load_wq(j)
            for c in range(chunks):
                xc = xcs.pop(c)
                for j in range(NG):
                    front_group(c, j, xc)
                    if c >= 2:
                        for k in range(nslice * j // NG, nslice * (j + 1) // NG):
                            tail_b_slice(c - 2, k)
                if c == 0:
                    load_late_consts()
                if c + 1 < chunks:
                    xcs[c + 1] = load_xc(c + 1)
                if c >= 1:
                    stage_tail_a(c - 1)
            stage_tail_a(chunks - 1)
            for cc in (chunks - 2, chunks - 1):
                if cc >= 0 and cc in ystash:
                    for k in range(nslice):
                        tail_b_slice(cc, k)

            # --- final states: last column of each scan ---
            fin = const.tile([128, 16], bf16)
            for s, tag in enumerate(("rkvf", "rksf", "rkvs", "rkss")):
                for j in range(NG):
                    nc.vector.tensor_copy(fin[:, 4 * s + j:4 * s + j + 1],
                                          prev[tag][j][:, tc - 1:tc])
            nc.sync.dma_start(out=finals, in_=fin)

    nc.compile()
    return nc


def _host_inputs(x, w_qkv, w_out, alpha, beta, t=T, tc=TC):
    """Build the 8 per-core input maps (host-side shard + transpose + cast)."""
    x = np.asarray(x, dtype=np.float32)
    w_qkv = np.asarray(w_qkv, dtype=np.float32)
    w_out = np.asarray(w_out, dtype=np.float32)
    alpha = np.asarray(alpha, dtype=np.float32)
    beta = np.asarray(beta, dtype=np.float32)

    d_a = 1.0 / (1.0 + np.exp(-alpha.astype(np.float64)))
    d_b = 1.0 / (1.0 + np.exp(-beta.astype(np.float64)))
    d_a = d_a.astype(np.float32)
    d_b = d_b.astype(np.float32)

    esel = np.zeros((2 * NG, 128, 128), dtype=BF16)
    ehead = np.zeros((2 * NG, 128, 128), dtype=BF16)
    for s in range(2):
        for j in range(NG):
            r = 32 * j + 2 * s
            esel[4 * s + j, r, 0:64] = 1
            esel[4 * s + j, r + 1, 64:128] = 1
            ehead[4 * s + j, 0:64, r] = 1
            ehead[4 * s + j, 64:128, r + 1] = 1

    in_maps = []
    for core in range(NCORES):
        b, g = core // 2, core % 2
        xt = np.ascontiguousarray(x[b, :t].T).reshape(8, 128, t).astype(BF16)
        blocks = []
        for j in range(NG):
            for off in (0, 1024, 2048):
                c0 = off + 512 * g + 128 * j
                blocks.append(w_qkv[:, c0:c0 + 128])
        wqc = np.concatenate(blocks, axis=1).reshape(8, 128, 1536)
        wqc = np.ascontiguousarray(wqc).astype(BF16)
        fo = w_out[512 * g:512 * g + 512, :]
        so = w_out[1024 + 512 * g:1024 + 512 * g + 512, :]
        woc = np.concatenate([fo, so], axis=0).reshape(8, 128, 1024)
        woc = np.ascontiguousarray(woc).astype(BF16)

        deca = np.zeros((NG, 128, tc), dtype=np.float32)
        decb = np.zeros((NG, 128, tc), dtype=np.float32)
        for j in range(NG):
            deca[j, 0:64, :] = d_a[8 * g + 2 * j]
            deca[j, 64:128, :] = d_a[8 * g + 2 * j + 1]
            decb[j, 0:64, :] = d_b[8 * g + 2 * j]
            decb[j, 64:128, :] = d_b[8 * g + 2 * j + 1]

        in_maps.append({
            "xT": xt, "wq": wqc, "wo": woc,
            "deca": deca, "decb": decb,
            "esel": esel, "ehead": ehead,
        })
    return in_maps


def _assemble(results, t=T):
    out = np.zeros((B, t, HID), dtype=np.float32)
    kv_f1 = np.zeros((B, H, D), dtype=np.float32)
    ks_f1 = np.zeros((B, H, D), dtype=np.float32)
    kv_s1 = np.zeros((B, H, D), dtype=np.float32)
    ks_s1 = np.zeros((B, H, D), dtype=np.float32)
    for core in range(NCORES):
        b, g = core // 2, core % 2
        out[b] += results[core]["yout"].astype(np.float32)
        fin = results[core]["finals"].astype(np.float32)
        for s, arr in enumerate((kv_f1, ks_f1, kv_s1, ks_s1)):
            for j in range(NG):
                col = fin[:, 4 * s + j]
                arr[b, 8 * g + 2 * j, :] = col[0:64]
                arr[b, 8 * g + 2 * j + 1, :] = col[64:128]
    return out, (kv_f1, ks_f1, kv_s1, ks_s1)


def kernel(x, w_qkv, w_out, alpha, beta, _trace=False):
    key = (T, TC)
    if key not in _BUILD_CACHE:
        _BUILD_CACHE[key] = build(T, TC)
    nc = _BUILD_CACHE[key]
    in_maps = _host_inputs(x, w_qkv, w_out, alpha, beta, T, TC)
    kw = {}
    if _trace:
        kw["trace"] = True
    res = run_bass_kernel_spmd(nc, in_maps, list(range(NCORES)), **kw)
    outs = _assemble(res.results, T)
    if _trace:
        kernel.last_exec_time_ns = res.exec_time_ns
        kernel.last_result = res
    return outs


# revision 43
# speedup vs baseline: 1.2943x; 1.0128x over previous
"""Dual-state linear attention Trainium2 kernel (8 NeuronCores, SPMD).

Sharding: core = (batch b, head-group g): b = core // 2, g = core % 2.
Each core processes batch b and heads 8g..8g+7 (feature-sharded w_qkv /
w_out slices).  The out-projection partial sums of the two head groups of
each batch are added on the host.

On-chip layout: feature-on-partition, time-on-free ("transposed"); x is
pre-transposed and bf16-cast on the host so no on-device transposes exist.
 - qkv matmul: psum[j 128, t TC] = sum_a wq[k-tile, j-tile].T @ xT[k-tile, chunk]
 - phi(x) = elu(x)+1 = relu(x) + exp(min(x, 0)):
     rneg = Relu(-x) (ACT), e = Exp(-rneg) (ACT),
     phi  = (x max 0) + e   (DVE scalar_tensor_tensor, drains PSUM)
 - decay scans: DVE tensor_tensor_scan along the free (time) dim, fp32
     decay tiles (bf16 decay would distort 1/(1-d) by ~6%), chained
     across chunks via initial = prev[:, -1:]
 - den[h,t] = sum_d q*run_ks: selector matmuls accumulate both decays'
     head-sums into ONE psum bank at rows 32j + 2s + e
 - 1/den = Exp(-Ln(den + 1e-20)) on ACT (Reciprocal is blocked; unused
     rows are exact 0 -> finite garbage killed by 0 selector weights)
 - broadcast 1/den rows to 64-row blocks: selector matmul + ACT drain
 - Y = (q * run_kv) * den_inv_bcast (DVE, bf16 2x mode)
 - out[t, o] = sum_y Y[y-tile, t-sub].T @ wo[y-tile, o-bank] (PE),
     ACT-drained to bf16, DMA out; host sums the 2 head-group partials.

Scheduling: 2-deep software pipeline, interleaved at feature-group
granularity so the in-order PE queue can fill qkv stalls with the
out-projection of chunk c-2 (whose Y tiles are certainly ready):
  front(c) group j  ||  outproj slice j of chunk c-2, then tailA(c-1).
Measured ~404 us on silicon (PE busy ~331 us, DVE ~291 us of which the
four scans are ~151 us at the hardware's 2.09 cycles/element).
"""
import sys

sys.path.insert(0, "/opt/trn_rl_repo")

import numpy as np
import ml_dtypes

import concourse.bass as bass
import concourse.bacc as bacc
import concourse.tile as tile
from concourse import mybir
from concourse.bass_utils import run_bass_kernel_spmd

BF16 = ml_dtypes.bfloat16

B, T, HID, H, D = 4, 4096, 1024, 16, 64
NCORES = 8
TC = 512            # time-chunk
NG = 4              # feature partition-groups per core (8 heads x 64 = 512 rows)

_BUILD_CACHE = {}


def build(t=T, tc=TC):
    """Build the SPMD Bass program. Returns (nc, names)."""
    chunks = t // tc
    nc = bacc.Bacc("TRN2", target_bir_lowering=False, debug=False,
                   enable_asserts=False, num_devices=NCORES)
    f32, bf16 = mybir.dt.float32, mybir.dt.bfloat16

    xT = nc.dram_tensor("xT", [8, 128, t], bf16, kind="ExternalInput").ap()
    wq = nc.dram_tensor("wq", [8, 128, 1536], bf16, kind="ExternalInput").ap()
    wo = nc.dram_tensor("wo", [8, 128, 1024], bf16, kind="ExternalInput").ap()
    deca = nc.dram_tensor("deca", [NG, 128, tc], f32, kind="ExternalInput").ap()
    decb = nc.dram_tensor("decb", [NG, 128, tc], f32, kind="ExternalInput").ap()
    esel = nc.dram_tensor("esel", [2 * NG, 128, 128], bf16, kind="ExternalInput").ap()
    ehead = nc.dram_tensor("ehead", [2 * NG, 128, 128], bf16, kind="ExternalInput").ap()
    yout = nc.dram_tensor("yout", [t, 1024], bf16, kind="ExternalOutput").ap()
    finals = nc.dram_tensor("finals", [128, 16], bf16, kind="ExternalOutput").ap()

    Relu = mybir.ActivationFunctionType.Relu
    Exp = mybir.ActivationFunctionType.Exp
    Ln = mybir.ActivationFunctionType.Ln
    MUL = mybir.AluOpType.mult
    ADD = mybir.AluOpType.add
    MAX = mybir.AluOpType.max

    with tile.TileContext(nc) as tc_:
        import contextlib
        ctx = contextlib.ExitStack()
        with ctx:
            const = ctx.enter_context(tc_.tile_pool(name="const", bufs=1))
            px = ctx.enter_context(tc_.tile_pool(name="px", bufs=2))
            pact = ctx.enter_context(tc_.tile_pool(name="pact", bufs=2))
            pqk = ctx.enter_context(tc_.tile_pool(name="pqk", bufs=5))
            pscan = ctx.enter_context(tc_.tile_pool(name="pscan", bufs=8))
            pmid = ctx.enter_context(tc_.tile_pool(name="pmid", bufs=8))
            pbc = ctx.enter_context(tc_.tile_pool(name="pbc", bufs=3))
            py = ctx.enter_context(tc_.tile_pool(name="py", bufs=12))
            pden = ctx.enter_context(tc_.tile_pool(name="pden", bufs=2))
            pout = ctx.enter_context(tc_.tile_pool(name="pout", bufs=2))
            ps_q = ctx.enter_context(
                tc_.tile_pool(name="ps_q", bufs=2, space="PSUM"))
            ps_k = ctx.enter_context(
                tc_.tile_pool(name="ps_k", bufs=2, space="PSUM"))
            ps_v = ctx.enter_context(
                tc_.tile_pool(name="ps_v", bufs=1, space="PSUM"))
            ps_bc = ctx.enter_context(
                tc_.tile_pool(name="ps_bc", bufs=1, space="PSUM"))
            ps_out = ctx.enter_context(
                tc_.tile_pool(name="ps_out", bufs=2, space="PSUM"))

            # constants / weights resident in SBUF (emitted inside the
            # pipeline driver: xc(0) and group-0 blocks first)
            wq_sb = const.tile([128, 8, 12, 128], bf16)
            wqr = wq.rearrange("a p (blk n) -> a p blk n", n=128)

            def load_wq(j):
                for a in range(8):
                    nc.sync.dma_start(out=wq_sb[:, a, 3 * j:3 * j + 3, :],
                                      in_=wqr[a, :, 3 * j:3 * j + 3, :])
            deca_sb = const.tile([128, NG, tc], f32)
            decb_sb = const.tile([128, NG, tc], f32)

            def load_dec():
                for j in range(NG):
                    nc.sync.dma_start(out=deca_sb[:, j, :], in_=deca[j])
                    nc.sync.dma_start(out=decb_sb[:, j, :], in_=decb[j])
            ehead_sb = const.tile([128, 2 * NG, 128], bf16)
            esel_sb = const.tile([128, 2 * NG, 128], bf16)
            wo_sb = const.tile([128, 8, 1024], bf16)
            epsb = const.tile([128, 1], f32)
            nc.vector.memset(epsb, 1e-20)

            def load_late_consts():
                for j in range(2 * NG):
                    nc.sync.dma_start(out=ehead_sb[:, j, :], in_=ehead[j])
                    nc.sync.dma_start(out=esel_sb[:, j, :], in_=esel[j])
                for a in range(8):
                    nc.sync.dma_start(out=wo_sb[:, a, :], in_=wo[a])

            prev = {tag: [None] * NG
                    for tag in ("rkvf", "rkvs", "rksf", "rkss")}
            stash = {}   # chunk -> (tmpf_j, tmps_j, numf_j, nums_j)

            def load_xc(c):
                t0 = c * tc
                xc = px.tile([128, 8, tc], bf16, tag="xc")
                for a in range(8):
                    nc.sync.dma_start(out=xc[:, a, :], in_=xT[a, :, t0:t0 + tc])
                return xc

            def front_group(c, j, xc):
                """qkv matmuls + phi + kv + scans + tmp/num for (chunk, grp)."""
                if j == 0:
                    stash[c] = ([None] * NG, [None] * NG,
                                [None] * NG, [None] * NG)
                tmpf_j, tmps_j, numf_j, nums_j = stash[c]
                if True:
                    psq = ps_q.tile([128, tc], f32, tag="psq")
                    psk = ps_k.tile([128, tc], f32, tag="psk")
                    psv = ps_v.tile([128, tc], f32, tag="psv")
                    for qi, ps in enumerate((psq, psk, psv)):
                        for a in range(8):
                            nc.tensor.matmul(
                                ps, wq_sb[:, a, 3 * j + qi, :],
                                xc[:, a, :], start=(a == 0), stop=(a == 7))

                    qsb = pqk.tile([128, tc], bf16, tag="q")
                    ksb = pqk.tile([128, tc], bf16, tag="k")
                    for src, dst in ((psq, qsb), (psk, ksb)):
                        rneg = pact.tile([128, tc], f32, tag="rneg")
                        nc.scalar.activation(rneg, src, Relu, scale=-1.0)
                        ex = pact.tile([128, tc], f32, tag="ex")
                        nc.scalar.activation(ex, rneg, Exp, scale=-1.0)
                        nc.vector.scalar_tensor_tensor(
                            dst, src, 0.0, ex, MAX, ADD)

                    kvt = pqk.tile([128, tc], bf16, tag="kv")
                    nc.vector.tensor_mul(kvt, ksb, psv)

                    souts = {}
                    for tag, src, dsb in (
                            ("rkvf", kvt, deca_sb), ("rkvs", kvt, decb_sb),
                            ("rksf", ksb, deca_sb), ("rkss", ksb, decb_sb)):
                        so = pscan.tile([128, tc], bf16, tag=tag)
                        init = (0.0 if c == 0
                                else prev[tag][j][:, tc - 1:tc])
                        nc.vector.tensor_tensor_scan(
                            so, dsb[:, j, :], src, init, MUL, ADD)
                        prev[tag][j] = so
                        souts[tag] = so

                    tmpf = pmid.tile([128, tc], bf16, tag="tmpf")
                    nc.vector.tensor_mul(tmpf, qsb, souts["rksf"])
                    tmps = pmid.tile([128, tc], bf16, tag="tmps")
                    nc.vector.tensor_mul(tmps, qsb, souts["rkss"])
                    numf = pmid.tile([128, tc], bf16, tag="numf")
                    nc.vector.tensor_mul(numf, qsb, souts["rkvf"])
                    nums = pmid.tile([128, tc], bf16, tag="nums")
                    nc.vector.tensor_mul(nums, qsb, souts["rkvs"])
                    tmpf_j[j], tmps_j[j] = tmpf, tmps
                    numf_j[j], nums_j[j] = numf, nums

            def stage_tail_a(c):
                """den + 1/den + broadcast + Y for chunk c."""
                tmpf_j, tmps_j, numf_j, nums_j = stash.pop(c)
                yf_j, ys_j = [None] * NG, [None] * NG

                # both decays' denominators in ONE psum bank:
                # group j / decay s / head-parity e at row 32j + 2s + e
                dp = ps_bc.tile([128, tc], f32, tag="bc")
                for i, tm in enumerate(tmpf_j + tmps_j):
                    s, j = divmod(i, NG)
                    nc.tensor.matmul(dp, ehead_sb[:, 4 * s + j, :], tm,
                                     start=(i == 0), stop=(i == 2 * NG - 1))

                # 1/den = Exp(-Ln(den + 1e-20)); unused rows are exact 0,
                # Ln(1e-20) = -46 -> Exp(46) finite, killed by 0 selector.
                dinv = pden.tile([128, tc], bf16, tag="dinv")
                lnd = pact.tile([128, tc], f32, tag="lnd")
                nc.scalar.activation(lnd, dp, Ln, bias=epsb)
                nc.scalar.activation(dinv, lnd, Exp, scale=-1.0)

                def bcast_pe(s, j, tag):
                    """den_inv rows {32j+2s, 32j+2s+1} -> [128, tc] via
                    selector matmul + ACT drain."""
                    bc_ps = ps_bc.tile([128, tc], f32, tag="bc")
                    nc.tensor.matmul(bc_ps, esel_sb[:, 4 * s + j, :], dinv,
                                     start=True, stop=True)
                    bc = pbc.tile([128, tc], bf16, tag=tag)
                    nc.scalar.copy(bc, bc_ps)
                    return bc

                for j in range(NG):
                    bcf = bcast_pe(0, j, "bcf")
                    yf = py.tile([128, tc], bf16, tag="yf")
                    nc.vector.tensor_mul(yf, numf_j[j], bcf)
                    yf_j[j] = yf

                    bcs = bcast_pe(1, j, "bcs")
                    ys = py.tile([128, tc], bf16, tag="ys")
                    nc.vector.tensor_mul(ys, nums_j[j], bcs)
                    ys_j[j] = ys

                ystash[c] = yf_j + ys_j

            def tail_b_slice(c, ts_):
                """out-projection for t-subtile ts_ of chunk c."""
                t0 = c * tc
                ytiles = ystash[c]
                osb = pout.tile([128, 1024], bf16, tag="osb")
                for ob in range(2):
                    pso = ps_out.tile([128, 512], f32, tag="pso")
                    for yt in range(8):
                        nc.tensor.matmul(
                            pso,
                            ytiles[yt][:, 128 * ts_:128 * ts_ + 128],
                            wo_sb[:, yt, 512 * ob:512 * ob + 512],
                            start=(yt == 0), stop=(yt == 7))
                    nc.scalar.copy(osb[:, 512 * ob:512 * ob + 512], pso)
                nc.sync.dma_start(
                    out=yout[t0 + 128 * ts_:t0 + 128 * ts_ + 128, :],
                    in_=osb)
                if ts_ == tc // 128 - 1:
                    del ystash[c]

            ystash = {}
            # 2-deep software pipeline, group-interleaved:
            #   front(c) groups  ||  outproj slices of chunk c-2  ||  tailA(c-1)
            nslice = tc // 128
            load_wq(0)
            xcs = {0: load_xc(0)}
            load_dec()
            for j in range(1, NG):
                load_wq(j)
            for c in range(chunks):
                xc = xcs.pop(c)
                for j in range(NG):
                    front_group(c, j, xc)
                    if c >= 2:
                        for k in range(nslice * j // NG, nslice * (j + 1) // NG):
                            tail_b_slice(c - 2, k)
                if c == 0:
                    load_late_consts()
                if c + 1 < chunks:
                    xcs[c + 1] = load_xc(c + 1)
                if c >= 1:
                    stage_tail_a(c - 1)
            stage_tail_a(chunks - 1)
            for cc in (chunks - 2, chunks - 1):
                if cc >= 0 and cc in ystash:
                    for k in range(nslice):
                        tail_b_slice(cc, k)

            # --- final states: last column of each scan ---
            fin = const.tile([128, 16], bf16)
            for s, tag in enumerate(("rkvf", "rksf", "rkvs", "rkss")):
                for j in range(NG):
                    nc.vector.tensor_copy(fin[:, 4 * s + j:4 * s + j + 1],
                                          prev[tag][j][:, tc - 1:tc])
            nc.sync.dma_start(out=finals, in_=fin)

    nc.compile()
    return nc


def _host_inputs(x, w_qkv, w_out, alpha, beta, t=T, tc=TC):
    """Build the 8 per-core input maps (host-side shard + transpose + cast)."""
    x = np.asarray(x, dtype=np.float32)
    w_qkv = np.asarray(w_qkv, dtype=np.float32)
    w_out = np.asarray(w_out, dtype=np.float32)
    alpha = np.asarray(alpha, dtype=np.float32)
    beta = np.asarray(beta, dtype=np.float32)

    d_a = 1.0 / (1.0 + np.exp(-alpha.astype(np.float64)))
    d_b = 1.0 / (1.0 + np.exp(-beta.astype(np.float64)))
    d_a = d_a.astype(np.float32)
    d_b = d_b.astype(np.float32)

    esel = np.zeros((2 * NG, 128, 128), dtype=BF16)
    ehead = np.zeros((2 * NG, 128, 128), dtype=BF16)
    for s in range(2):
        for j in range(NG):
            r = 32 * j + 2 * s
            esel[4 * s + j, r, 0:64] = 1
            esel[4 * s + j, r + 1, 64:128] = 1
            ehead[4 * s + j, 0:64, r] = 1
            ehead[4 * s + j, 64:128, r + 1] = 1

    in_maps = []
    for core in range(NCORES):
        b, g = core // 2, core % 2
        xt = np.ascontiguousarray(x[b, :t].T).reshape(8, 128, t).astype(BF16)
        blocks = []
        for j in range(NG):
            for off in (0, 1024, 2048):
                c0 = off + 512 * g + 128 * j
                blocks.append(w_qkv[:, c0:c0 + 128])
        wqc = np.concatenate(blocks, axis=1).reshape(8, 128, 1536)
        wqc = np.ascontiguousarray(wqc).astype(BF16)
        fo = w_out[512 * g:512 * g + 512, :]
        so = w_out[1024 + 512 * g:1024 + 512 * g + 512, :]
        woc = np.concatenate([fo, so], axis=0).reshape(8, 128, 1024)
        woc = np.ascontiguousarray(woc).astype(BF16)

        deca = np.zeros((NG, 128, tc), dtype=np.float32)
        decb = np.zeros((NG, 128, tc), dtype=np.float32)
        for j in range(NG):
            deca[j, 0:64, :] = d_a[8 * g + 2 * j]
            deca[j, 64:128, :] = d_a[8 * g + 2 * j + 1]
            decb[j, 0:64, :] = d_b[8 * g + 2 * j]
            decb[j, 64:128, :] = d_b[8 * g + 2 * j + 1]

        in_maps.append({
            "xT": xt, "wq": wqc, "wo": woc,
            "deca": deca, "decb": decb,
            "esel": esel, "ehead": ehead,
        })
    return in_maps


def _assemble(results, t=T):
    out = np.zeros((B, t, HID), dtype=np.float32)
    kv_f1 = np.zeros((B, H, D), dtype=np.float32)
    ks_f1 = np.zeros((B, H, D), dtype=np.float32)
    kv_s1 = np.zeros((B, H, D), dtype=np.float32)
    ks_s1 = np.zeros((B, H, D), dtype=np.float32)
    for core in range(NCORES):
        b, g = core // 2, core % 2
        out[b] += results[core]["yout"].astype(np.float32)
        fin = results[core]["finals"].astype(np.float32)
        for s, arr in enumerate((kv_f1, ks_f1, kv_s1, ks_s1)):
            for j in range(NG):
                col = fin[:, 4 * s + j]
                arr[b, 8 * g + 2 * j, :] = col[0:64]
                arr[b, 8 * g + 2 * j + 1, :] = col[64:128]
    return out, (kv_f1, ks_f1, kv_s1, ks_s1)


def kernel(x, w_qkv, w_out, alpha, beta, _trace=False):
    key = (T, TC)
    if key not in _BUILD_CACHE:
        _BUILD_CACHE[key] = build(T, TC)
    nc = _BUILD_CACHE[key]
    in_maps = _host_inputs(x, w_qkv, w_out, alpha, beta, T, TC)
    kw = {}
    if _trace:
        kw["trace"] = True
    res = run_bass_kernel_spmd(nc, in_maps, list(range(NCORES)), **kw)
    outs = _assemble(res.results, T)
    if _trace:
        kernel.last_exec_time_ns = res.exec_time_ns
        kernel.last_result = res
    return outs


# revision 47
# speedup vs baseline: 1.3516x; 1.0442x over previous
"""Dual-state linear attention Trainium2 kernel (8 NeuronCores, SPMD).

Sharding: core = (batch b, head-group g): b = core // 2, g = core % 2.
Each core processes batch b and heads 8g..8g+7 (feature-sharded w_qkv /
w_out slices).  The out-projection partial sums of the two head groups of
each batch are added on the host.

On-chip layout: feature-on-partition, time-on-free ("transposed"); x is
pre-transposed and bf16-cast on the host so no on-device transposes exist.
 - qkv matmul: psum[j 128, t TC] = sum_a wq[k-tile, j-tile].T @ xT[k-tile, chunk]
 - phi(x) = elu(x)+1 = relu(x) + exp(min(x, 0)):
     rneg = Relu(-x) (ACT), e = Exp(-rneg) (ACT),
     phi  = (x max 0) + e   (DVE scalar_tensor_tensor, drains PSUM)
 - decay scans: DVE tensor_tensor_scan along the free (time) dim, fp32
     decay tiles (bf16 decay would distort 1/(1-d) by ~6%), chained
     across chunks via initial = prev[:, -1:]
 - den[h,t] = sum_d q*run_ks: selector matmuls accumulate both decays'
     head-sums into ONE psum bank at rows 32j + 2s + e
 - 1/den = Exp(-Ln(den + 1e-20)) on ACT (Reciprocal is blocked; unused
     rows are exact 0 -> finite garbage killed by 0 selector weights)
 - broadcast 1/den rows to 64-row blocks: selector matmul + ACT drain
 - Y = (q * run_kv) * den_inv_bcast (DVE, bf16 2x mode)
 - out[t, o] = sum_y Y[y-tile, t-sub].T @ wo[y-tile, o-bank] (PE),
     ACT-drained to bf16, DMA out; host sums the 2 head-group partials.

Scheduling: 2-deep software pipeline, interleaved at feature-group
granularity so the in-order PE queue can fill qkv stalls with the
out-projection of chunk c-2 (whose Y tiles are certainly ready):
  front(c) group j  ||  outproj slice j of chunk c-2, then tailA(c-1).
Measured ~396 us on silicon (PE busy ~331 us, DVE ~291 us of which the
four scans are ~151 us at the hardware's 2.09 cycles/element).
"""
import sys

sys.path.insert(0, "/opt/trn_rl_repo")

import numpy as np
import ml_dtypes

import concourse.bass as bass
import concourse.bacc as bacc
import concourse.tile as tile
from concourse import mybir
from concourse.bass_utils import run_bass_kernel_spmd

BF16 = ml_dtypes.bfloat16

B, T, HID, H, D = 4, 4096, 1024, 16, 64
NCORES = 8
TC = 512            # time-chunk
NG = 4              # feature partition-groups per core (8 heads x 64 = 512 rows)

_BUILD_CACHE = {}


def build(t=T, tc=TC):
    """Build the SPMD Bass program. Returns (nc, names)."""
    chunks = t // tc
    nc = bacc.Bacc("TRN2", target_bir_lowering=False, debug=False,
                   enable_asserts=False, num_devices=NCORES)
    f32, bf16 = mybir.dt.float32, mybir.dt.bfloat16

    xT = nc.dram_tensor("xT", [8, 128, t], bf16, kind="ExternalInput").ap()
    wq = nc.dram_tensor("wq", [8, 128, 1536], bf16, kind="ExternalInput").ap()
    wo = nc.dram_tensor("wo", [8, 128, 1024], bf16, kind="ExternalInput").ap()
    deca = nc.dram_tensor("deca", [NG, 128, tc], f32, kind="ExternalInput").ap()
    decb = nc.dram_tensor("decb", [NG, 128, tc], f32, kind="ExternalInput").ap()
    esel = nc.dram_tensor("esel", [2 * NG, 128, 128], bf16, kind="ExternalInput").ap()
    ehead = nc.dram_tensor("ehead", [2 * NG, 128, 128], bf16, kind="ExternalInput").ap()
    yout = nc.dram_tensor("yout", [t, 1024], bf16, kind="ExternalOutput").ap()
    finals = nc.dram_tensor("finals", [128, 16], bf16, kind="ExternalOutput").ap()

    Relu = mybir.ActivationFunctionType.Relu
    Exp = mybir.ActivationFunctionType.Exp
    Ln = mybir.ActivationFunctionType.Ln
    MUL = mybir.AluOpType.mult
    ADD = mybir.AluOpType.add
    MAX = mybir.AluOpType.max

    with tile.TileContext(nc) as tc_:
        import contextlib
        ctx = contextlib.ExitStack()
        with ctx:
            const = ctx.enter_context(tc_.tile_pool(name="const", bufs=1))
            px = ctx.enter_context(tc_.tile_pool(name="px", bufs=2))
            pact = ctx.enter_context(tc_.tile_pool(name="pact", bufs=2))
            pqk = ctx.enter_context(tc_.tile_pool(name="pqk", bufs=5))
            pscan = ctx.enter_context(tc_.tile_pool(name="pscan", bufs=8))
            pmid = ctx.enter_context(tc_.tile_pool(name="pmid", bufs=8))
            pbc = ctx.enter_context(tc_.tile_pool(name="pbc", bufs=3))
            py = ctx.enter_context(tc_.tile_pool(name="py", bufs=12))
            pden = ctx.enter_context(tc_.tile_pool(name="pden", bufs=2))
            pout = ctx.enter_context(tc_.tile_pool(name="pout", bufs=2))
            ps_q = ctx.enter_context(
                tc_.tile_pool(name="ps_q", bufs=2, space="PSUM"))
            ps_k = ctx.enter_context(
                tc_.tile_pool(name="ps_k", bufs=1, space="PSUM"))
            ps_v = ctx.enter_context(
                tc_.tile_pool(name="ps_v", bufs=1, space="PSUM"))
            ps_bc = ctx.enter_context(
                tc_.tile_pool(name="ps_bc", bufs=2, space="PSUM"))
            ps_out = ctx.enter_context(
                tc_.tile_pool(name="ps_out", bufs=2, space="PSUM"))

            # constants / weights resident in SBUF (emitted inside the
            # pipeline driver: xc(0) and group-0 blocks first)
            wq_sb = const.tile([128, 8, 12, 128], bf16)
            wqr = wq.rearrange("a p (blk n) -> a p blk n", n=128)

            def load_wq(j):
                for a in range(8):
                    nc.sync.dma_start(out=wq_sb[:, a, 3 * j:3 * j + 3, :],
                                      in_=wqr[a, :, 3 * j:3 * j + 3, :])
            deca_sb = const.tile([128, NG, tc], f32)
            decb_sb = const.tile([128, NG, tc], f32)

            def load_dec():
                for j in range(NG):
                    nc.sync.dma_start(out=deca_sb[:, j, :], in_=deca[j])
                    nc.sync.dma_start(out=decb_sb[:, j, :], in_=decb[j])
            ehead_sb = const.tile([128, 2 * NG, 128], bf16)
            esel_sb = const.tile([128, 2 * NG, 128], bf16)
            wo_sb = const.tile([128, 8, 1024], bf16)
            epsb = const.tile([128, 1], f32)
            nc.vector.memset(epsb, 1e-20)

            def load_late_consts():
                for j in range(2 * NG):
                    nc.sync.dma_start(out=ehead_sb[:, j, :], in_=ehead[j])
                    nc.sync.dma_start(out=esel_sb[:, j, :], in_=esel[j])
                for a in range(8):
                    nc.sync.dma_start(out=wo_sb[:, a, :], in_=wo[a])

            prev = {tag: [None] * NG
                    for tag in ("rkvf", "rkvs", "rksf", "rkss")}
            stash = {}   # chunk -> (tmpf_j, tmps_j, numf_j, nums_j)

            def load_xc(c):
                t0 = c * tc
                xc = px.tile([128, 8, tc], bf16, tag="xc")
                for a in range(8):
                    nc.sync.dma_start(out=xc[:, a, :], in_=xT[a, :, t0:t0 + tc])
                return xc

            def front_group(c, j, xc):
                """qkv matmuls + phi + kv + scans + tmp/num for (chunk, grp)."""
                if j == 0:
                    stash[c] = ([None] * NG, [None] * NG,
                                [None] * NG, [None] * NG)
                tmpf_j, tmps_j, numf_j, nums_j = stash[c]
                if True:
                    psq = ps_q.tile([128, tc], f32, tag="psq")
                    psk = ps_k.tile([128, tc], f32, tag="psk")
                    psv = ps_v.tile([128, tc], f32, tag="psv")
                    for qi, ps in enumerate((psq, psk, psv)):
                        for a in range(8):
                            nc.tensor.matmul(
                                ps, wq_sb[:, a, 3 * j + qi, :],
                                xc[:, a, :], start=(a == 0), stop=(a == 7))

                    qsb = pqk.tile([128, tc], bf16, tag="q")
                    ksb = pqk.tile([128, tc], bf16, tag="k")
                    for src, dst in ((psq, qsb), (psk, ksb)):
                        rneg = pact.tile([128, tc], f32, tag="rneg")
                        nc.scalar.activation(rneg, src, Relu, scale=-1.0)
                        ex = pact.tile([128, tc], f32, tag="ex")
                        nc.scalar.activation(ex, rneg, Exp, scale=-1.0)
                        nc.vector.scalar_tensor_tensor(
                            dst, src, 0.0, ex, MAX, ADD)

                    kvt = pqk.tile([128, tc], bf16, tag="kv")
                    nc.vector.tensor_mul(kvt, ksb, psv)

                    souts = {}
                    for tag, src, dsb in (
                            ("rkvf", kvt, deca_sb), ("rkvs", kvt, decb_sb),
                            ("rksf", ksb, deca_sb), ("rkss", ksb, decb_sb)):
                        so = pscan.tile([128, tc], bf16, tag=tag)
                        init = (0.0 if c == 0
                                else prev[tag][j][:, tc - 1:tc])
                        nc.vector.tensor_tensor_scan(
                            so, dsb[:, j, :], src, init, MUL, ADD)
                        prev[tag][j] = so
                        souts[tag] = so

                    tmpf = pmid.tile([128, tc], bf16, tag="tmpf")
                    nc.vector.tensor_mul(tmpf, qsb, souts["rksf"])
                    tmps = pmid.tile([128, tc], bf16, tag="tmps")
                    nc.vector.tensor_mul(tmps, qsb, souts["rkss"])
                    numf = pmid.tile([128, tc], bf16, tag="numf")
                    nc.vector.tensor_mul(numf, qsb, souts["rkvf"])
                    nums = pmid.tile([128, tc], bf16, tag="nums")
                    nc.vector.tensor_mul(nums, qsb, souts["rkvs"])
                    tmpf_j[j], tmps_j[j] = tmpf, tmps
                    numf_j[j], nums_j[j] = numf, nums

            def stage_tail_a(c):
                """den + 1/den + broadcast + Y for chunk c."""
                tmpf_j, tmps_j, numf_j, nums_j = stash.pop(c)
                yf_j, ys_j = [None] * NG, [None] * NG

                # both decays' denominators in ONE psum bank:
                # group j / decay s / head-parity e at row 32j + 2s + e
                dp = ps_bc.tile([128, tc], f32, tag="bc")
                for i, tm in enumerate(tmpf_j + tmps_j):
                    s, j = divmod(i, NG)
                    nc.tensor.matmul(dp, ehead_sb[:, 4 * s + j, :], tm,
                                     start=(i == 0), stop=(i == 2 * NG - 1))

                # 1/den = Exp(-Ln(den + 1e-20)); unused rows are exact 0,
                # Ln(1e-20) = -46 -> Exp(46) finite, killed by 0 selector.
                dinv = pden.tile([128, tc], bf16, tag="dinv")
                lnd = pact.tile([128, tc], f32, tag="lnd")
                nc.scalar.activation(lnd, dp, Ln, bias=epsb)
                nc.scalar.activation(dinv, lnd, Exp, scale=-1.0)

                def bcast_pe(s, j, tag):
                    """den_inv rows {32j+2s, 32j+2s+1} -> [128, tc] via
                    selector matmul + ACT drain."""
                    bc_ps = ps_bc.tile([128, tc], f32, tag="bc")
                    nc.tensor.matmul(bc_ps, esel_sb[:, 4 * s + j, :], dinv,
                                     start=True, stop=True)
                    bc = pbc.tile([128, tc], bf16, tag=tag)
                    nc.scalar.copy(bc, bc_ps)
                    return bc

                for j in range(NG):
                    bcf = bcast_pe(0, j, "bcf")
                    yf = py.tile([128, tc], bf16, tag="yf")
                    nc.vector.tensor_mul(yf, numf_j[j], bcf)
                    yf_j[j] = yf

                    bcs = bcast_pe(1, j, "bcs")
                    ys = py.tile([128, tc], bf16, tag="ys")
                    nc.vector.tensor_mul(ys, nums_j[j], bcs)
                    ys_j[j] = ys

                ystash[c] = yf_j + ys_j

            def tail_b_slice(c, ts_):
                """out-projection for t-subtile ts_ of chunk c."""
                t0 = c * tc
                ytiles = ystash[c]
                osb = pout.tile([128, 1024], bf16, tag="osb")
                for ob in range(2):
                    pso = ps_out.tile([128, 512], f32, tag="pso")
                    for yt in range(8):
                        nc.tensor.matmul(
                            pso,
                            ytiles[yt][:, 128 * ts_:128 * ts_ + 128],
                            wo_sb[:, yt, 512 * ob:512 * ob + 512],
                            start=(yt == 0), stop=(yt == 7))
                    nc.scalar.copy(osb[:, 512 * ob:512 * ob + 512], pso)
                nc.sync.dma_start(
                    out=yout[t0 + 128 * ts_:t0 + 128 * ts_ + 128, :],
                    in_=osb)
                if ts_ == tc // 128 - 1:
                    del ystash[c]

            ystash = {}
            # 2-deep software pipeline, group-interleaved:
            #   front(c) groups  ||  outproj slices of chunk c-2  ||  tailA(c-1)
            nslice = tc // 128
            load_wq(0)
            xcs = {0: load_xc(0)}
            load_dec()
            for j in range(1, NG):
                load_wq(j)
            for c in range(chunks):
                xc = xcs.pop(c)
                for j in range(NG):
                    front_group(c, j, xc)
                    if c >= 2:
                        for k in range(nslice * j // NG, nslice * (j + 1) // NG):
                            tail_b_slice(c - 2, k)
                if c == 0:
                    load_late_consts()
                if c + 1 < chunks:
                    xcs[c + 1] = load_xc(c + 1)
                if c >= 1:
                    stage_tail_a(c - 1)
            stage_tail_a(chunks - 1)
            for cc in (chunks - 2, chunks - 1):
                if cc >= 0 and cc in ystash:
                    for k in range(nslice):
                        tail_b_slice(cc, k)

            # --- final states: last column of each scan ---
            fin = const.tile([128, 16], bf16)
            for s, tag in enumerate(("rkvf", "rksf", "rkvs", "rkss")):
                for j in range(NG):
                    nc.vector.tensor_copy(fin[:, 4 * s + j:4 * s + j + 1],
                                          prev[tag][j][:, tc - 1:tc])
            nc.sync.dma_start(out=finals, in_=fin)

    nc.compile()
    return nc


def _host_inputs(x, w_qkv, w_out, alpha, beta, t=T, tc=TC):
    """Build the 8 per-core input maps (host-side shard + transpose + cast)."""
    x = np.asarray(x, dtype=np.float32)
    w_qkv = np.asarray(w_qkv, dtype=np.float32)
    w_out = np.asarray(w_out, dtype=np.float32)
    alpha = np.asarray(alpha, dtype=np.float32)
    beta = np.asarray(beta, dtype=np.float32)

    d_a = 1.0 / (1.0 + np.exp(-alpha.astype(np.float64)))
    d_b = 1.0 / (1.0 + np.exp(-beta.astype(np.float64)))
    d_a = d_a.astype(np.float32)
    d_b = d_b.astype(np.float32)

    esel = np.zeros((2 * NG, 128, 128), dtype=BF16)
    ehead = np.zeros((2 * NG, 128, 128), dtype=BF16)
    for s in range(2):
        for j in range(NG):
            r = 32 * j + 2 * s
            esel[4 * s + j, r, 0:64] = 1
            esel[4 * s + j, r + 1, 64:128] = 1
            ehead[4 * s + j, 0:64, r] = 1
            ehead[4 * s + j, 64:128, r + 1] = 1

    in_maps = []
    for core in range(NCORES):
        b, g = core // 2, core % 2
        xt = np.ascontiguousarray(x[b, :t].T).reshape(8, 128, t).astype(BF16)
        blocks = []
        for j in range(NG):
            for off in (0, 1024, 2048):
                c0 = off + 512 * g + 128 * j
                blocks.append(w_qkv[:, c0:c0 + 128])
        wqc = np.concatenate(blocks, axis=1).reshape(8, 128, 1536)
        wqc = np.ascontiguousarray(wqc).astype(BF16)
        fo = w_out[512 * g:512 * g + 512, :]
        so = w_out[1024 + 512 * g:1024 + 512 * g + 512, :]
        woc = np.concatenate([fo, so], axis=0).reshape(8, 128, 1024)
        woc = np.ascontiguousarray(woc).astype(BF16)

        deca = np.zeros((NG, 128, tc), dtype=np.float32)
        decb = np.zeros((NG, 128, tc), dtype=np.float32)
        for j in range(NG):
            deca[j, 0:64, :] = d_a[8 * g + 2 * j]
            deca[j, 64:128, :] = d_a[8 * g + 2 * j + 1]
            decb[j, 0:64, :] = d_b[8 * g + 2 * j]
            decb[j, 64:128, :] = d_b[8 * g + 2 * j + 1]

        in_maps.append({
            "xT": xt, "wq": wqc, "wo": woc,
            "deca": deca, "decb": decb,
            "esel": esel, "ehead": ehead,
        })
    return in_maps


def _assemble(results, t=T):
    out = np.zeros((B, t, HID), dtype=np.float32)
    kv_f1 = np.zeros((B, H, D), dtype=np.float32)
    ks_f1 = np.zeros((B, H, D), dtype=np.float32)
    kv_s1 = np.zeros((B, H, D), dtype=np.float32)
    ks_s1 = np.zeros((B, H, D), dtype=np.float32)
    for core in range(NCORES):
        b, g = core // 2, core % 2
        out[b] += results[core]["yout"].astype(np.float32)
        fin = results[core]["finals"].astype(np.float32)
        for s, arr in enumerate((kv_f1, ks_f1, kv_s1, ks_s1)):
            for j in range(NG):
                col = fin[:, 4 * s + j]
                arr[b, 8 * g + 2 * j, :] = col[0:64]
                arr[b, 8 * g + 2 * j + 1, :] = col[64:128]
    return out, (kv_f1, ks_f1, kv_s1, ks_s1)


def kernel(x, w_qkv, w_out, alpha, beta, _trace=False):
    key = (T, TC)
    if key not in _BUILD_CACHE:
        _BUILD_CACHE[key] = build(T, TC)
    nc = _BUILD_CACHE[key]
    in_maps = _host_inputs(x, w_qkv, w_out, alpha, beta, T, TC)
    kw = {}
    if _trace:
        kw["trace"] = True
    res = run_bass_kernel_spmd(nc, in_maps, list(range(NCORES)), **kw)
    outs = _assemble(res.results, T)
    if _trace:
        kernel.last_exec_time_ns = res.exec_time_ns
        kernel.last_result = res
    return outs


# revision 52
# speedup vs baseline: 1.3623x; 1.0079x over previous
"""Dual-state linear attention Trainium2 kernel (8 NeuronCores, SPMD).

Sharding: core = (batch b, head-group g): b = core // 2, g = core % 2.
Each core processes batch b and heads 8g..8g+7 (feature-sharded w_qkv /
w_out slices).  The out-projection partial sums of the two head groups of
each batch are added on the host.

On-chip layout: feature-on-partition, time-on-free ("transposed"); x is
pre-transposed and bf16-cast on the host so no on-device transposes exist.
 - qkv matmul: psum[j 128, t TC] = sum_a wq[k-tile, j-tile].T @ xT[k-tile, chunk]
 - phi(x) = elu(x)+1 = relu(x) + exp(min(x, 0)):
     rneg = Relu(-x) (ACT), e = Exp(-rneg) (ACT),
     phi  = (x max 0) + e   (DVE scalar_tensor_tensor, drains PSUM)
 - decay scans: DVE tensor_tensor_scan along the free (time) dim, fp32
     decay tiles (bf16 decay would distort 1/(1-d) by ~6%), chained
     across chunks via initial = prev[:, -1:]
 - den[h,t] = sum_d q*run_ks: selector matmuls accumulate both decays'
     head-sums into ONE psum bank at rows 32j + 2s + e
 - 1/den = Exp(-Ln(den + 1e-20)) on ACT (Reciprocal is blocked; unused
     rows are exact 0 -> finite garbage killed by 0 selector weights)
 - broadcast 1/den rows to 64-row blocks: selector matmul + ACT drain
 - Y = (q * run_kv) * den_inv_bcast (DVE, bf16 2x mode)
 - out[t, o] = sum_y Y[y-tile, t-sub].T @ wo[y-tile, o-bank] (PE),
     ACT-drained to bf16, DMA out; host sums the 2 head-group partials.

Scheduling: 2-deep software pipeline, interleaved at feature-group
granularity so the in-order PE queue can fill qkv stalls with the
out-projection of chunk c-2 (whose Y tiles are certainly ready):
  front(c) group j  ||  outproj slice j of chunk c-2, then tailA(c-1).
Measured ~379 us on silicon (PE busy ~331 us, DVE ~291 us of which the
four scans are ~151 us at the hardware's 2.09 cycles/element).
"""
import sys

sys.path.insert(0, "/opt/trn_rl_repo")

import numpy as np
import ml_dtypes

import concourse.bass as bass
import concourse.bacc as bacc
import concourse.tile as tile
from concourse import mybir
from concourse.bass_utils import run_bass_kernel_spmd

BF16 = ml_dtypes.bfloat16

B, T, HID, H, D = 4, 4096, 1024, 16, 64
NCORES = 8
TC = 512            # time-chunk
NG = 4              # feature partition-groups per core (8 heads x 64 = 512 rows)

_BUILD_CACHE = {}


def build(t=T, tc=TC):
    """Build the SPMD Bass program. Returns (nc, names)."""
    chunks = t // tc
    nc = bacc.Bacc("TRN2", target_bir_lowering=False, debug=False,
                   enable_asserts=False, num_devices=NCORES)
    f32, bf16 = mybir.dt.float32, mybir.dt.bfloat16

    xT = nc.dram_tensor("xT", [8, 128, t], bf16, kind="ExternalInput").ap()
    wq = nc.dram_tensor("wq", [8, 128, 1536], bf16, kind="ExternalInput").ap()
    wo = nc.dram_tensor("wo", [8, 128, 1024], bf16, kind="ExternalInput").ap()
    deca = nc.dram_tensor("deca", [NG, 128, tc], f32, kind="ExternalInput").ap()
    decb = nc.dram_tensor("decb", [NG, 128, tc], f32, kind="ExternalInput").ap()
    esel = nc.dram_tensor("esel", [2 * NG, 128, 128], bf16, kind="ExternalInput").ap()
    ehead = nc.dram_tensor("ehead", [2 * NG, 128, 128], bf16, kind="ExternalInput").ap()
    yout = nc.dram_tensor("yout", [t, 1024], bf16, kind="ExternalOutput").ap()
    finals = nc.dram_tensor("finals", [128, 16], bf16, kind="ExternalOutput").ap()

    Relu = mybir.ActivationFunctionType.Relu
    Exp = mybir.ActivationFunctionType.Exp
    Ln = mybir.ActivationFunctionType.Ln
    MUL = mybir.AluOpType.mult
    ADD = mybir.AluOpType.add
    MAX = mybir.AluOpType.max

    with tile.TileContext(nc) as tc_:
        import contextlib
        ctx = contextlib.ExitStack()
        with ctx:
            const = ctx.enter_context(tc_.tile_pool(name="const", bufs=1))
            px = ctx.enter_context(tc_.tile_pool(name="px", bufs=2))
            pact = ctx.enter_context(tc_.tile_pool(name="pact", bufs=2))
            pqk = ctx.enter_context(tc_.tile_pool(name="pqk", bufs=5))
            pscan = ctx.enter_context(tc_.tile_pool(name="pscan", bufs=8))
            pmid = ctx.enter_context(tc_.tile_pool(name="pmid", bufs=8))
            pbc = ctx.enter_context(tc_.tile_pool(name="pbc", bufs=3))
            pv = ctx.enter_context(tc_.tile_pool(name="pv", bufs=2))
            py = ctx.enter_context(tc_.tile_pool(name="py", bufs=12))
            pden = ctx.enter_context(tc_.tile_pool(name="pden", bufs=2))
            pout = ctx.enter_context(tc_.tile_pool(name="pout", bufs=2))
            ps_q = ctx.enter_context(
                tc_.tile_pool(name="ps_q", bufs=2, space="PSUM"))
            ps_k = ctx.enter_context(
                tc_.tile_pool(name="ps_k", bufs=1, space="PSUM"))
            ps_v = ctx.enter_context(
                tc_.tile_pool(name="ps_v", bufs=1, space="PSUM"))
            ps_bc = ctx.enter_context(
                tc_.tile_pool(name="ps_bc", bufs=2, space="PSUM"))
            ps_out = ctx.enter_context(
                tc_.tile_pool(name="ps_out", bufs=2, space="PSUM"))

            # constants / weights resident in SBUF (emitted inside the
            # pipeline driver: xc(0) and group-0 blocks first)
            wq_sb = const.tile([128, 8, 12, 128], bf16)
            wqr = wq.rearrange("a p (blk n) -> a p blk n", n=128)

            def load_wq(j):
                for a in range(8):
                    nc.sync.dma_start(out=wq_sb[:, a, 3 * j:3 * j + 3, :],
                                      in_=wqr[a, :, 3 * j:3 * j + 3, :])
            deca_sb = const.tile([128, NG, tc], f32)
            decb_sb = const.tile([128, NG, tc], f32)

            def load_dec():
                for j in range(NG):
                    nc.sync.dma_start(out=deca_sb[:, j, :], in_=deca[j])
                    nc.sync.dma_start(out=decb_sb[:, j, :], in_=decb[j])
            ehead_sb = const.tile([128, 2 * NG, 128], bf16)
            esel_sb = const.tile([128, 2 * NG, 128], bf16)
            wo_sb = const.tile([128, 8, 1024], bf16)
            epsb = const.tile([128, 1], f32)
            nc.vector.memset(epsb, 1e-20)

            def load_late_consts():
                for j in range(2 * NG):
                    nc.sync.dma_start(out=ehead_sb[:, j, :], in_=ehead[j])
                    nc.sync.dma_start(out=esel_sb[:, j, :], in_=esel[j])
                for a in range(8):
                    nc.sync.dma_start(out=wo_sb[:, a, :], in_=wo[a])

            prev = {tag: [None] * NG
                    for tag in ("rkvf", "rkvs", "rksf", "rkss")}
            stash = {}   # chunk -> (tmpf_j, tmps_j, numf_j, nums_j)

            def load_xc(c):
                t0 = c * tc
                xc = px.tile([128, 8, tc], bf16, tag="xc")
                for a in range(8):
                    nc.sync.dma_start(out=xc[:, a, :], in_=xT[a, :, t0:t0 + tc])
                return xc

            def front_group(c, j, xc):
                """qkv matmuls + phi + kv + scans + tmp/num for (chunk, grp)."""
                if j == 0:
                    stash[c] = ([None] * NG, [None] * NG,
                                [None] * NG, [None] * NG)
                tmpf_j, tmps_j, numf_j, nums_j = stash[c]
                if True:
                    psq = ps_q.tile([128, tc], f32, tag="psq")
                    psk = ps_k.tile([128, tc], f32, tag="psk")
                    psv = ps_v.tile([128, tc], f32, tag="psv")
                    for qi, ps in enumerate((psq, psk, psv)):
                        for a in range(8):
                            nc.tensor.matmul(
                                ps, wq_sb[:, a, 3 * j + qi, :],
                                xc[:, a, :], start=(a == 0), stop=(a == 7))

                    qsb = pqk.tile([128, tc], bf16, tag="q")
                    ksb = pqk.tile([128, tc], bf16, tag="k")
                    for src, dst in ((psq, qsb), (psk, ksb)):
                        rneg = pact.tile([128, tc], f32, tag="rneg")
                        nc.scalar.activation(rneg, src, Relu, scale=-1.0)
                        ex = pact.tile([128, tc], f32, tag="ex")
                        nc.scalar.activation(ex, rneg, Exp, scale=-1.0)
                        nc.vector.scalar_tensor_tensor(
                            dst, src, 0.0, ex, MAX, ADD)

                    vsb = pv.tile([128, tc], bf16, tag="v")
                    nc.scalar.copy(vsb, psv)
                    kvt = pqk.tile([128, tc], bf16, tag="kv")
                    nc.vector.tensor_mul(kvt, ksb, vsb)

                    souts = {}
                    for tag, src, dsb in (
                            ("rkvf", kvt, deca_sb), ("rkvs", kvt, decb_sb),
                            ("rksf", ksb, deca_sb), ("rkss", ksb, decb_sb)):
                        so = pscan.tile([128, tc], bf16, tag=tag)
                        init = (0.0 if c == 0
                                else prev[tag][j][:, tc - 1:tc])
                        nc.vector.tensor_tensor_scan(
                            so, dsb[:, j, :], src, init, MUL, ADD)
                        prev[tag][j] = so
                        souts[tag] = so

                    tmpf = pmid.tile([128, tc], bf16, tag="tmpf")
                    nc.vector.tensor_mul(tmpf, qsb, souts["rksf"])
                    tmps = pmid.tile([128, tc], bf16, tag="tmps")
                    nc.vector.tensor_mul(tmps, qsb, souts["rkss"])
                    numf = pmid.tile([128, tc], bf16, tag="numf")
                    nc.vector.tensor_mul(numf, qsb, souts["rkvf"])
                    nums = pmid.tile([128, tc], bf16, tag="nums")
                    nc.vector.tensor_mul(nums, qsb, souts["rkvs"])
                    tmpf_j[j], tmps_j[j] = tmpf, tmps
                    numf_j[j], nums_j[j] = numf, nums

            def stage_tail_a(c):
                """den + 1/den + broadcast + Y for chunk c."""
                tmpf_j, tmps_j, numf_j, nums_j = stash.pop(c)
                yf_j, ys_j = [None] * NG, [None] * NG

                # both decays' denominators in ONE psum bank:
                # group j / decay s / head-parity e at row 32j + 2s + e
                dp = ps_bc.tile([128, tc], f32, tag="bc")
                for i, tm in enumerate(tmpf_j + tmps_j):
                    s, j = divmod(i, NG)
                    nc.tensor.matmul(dp, ehead_sb[:, 4 * s + j, :], tm,
                                     start=(i == 0), stop=(i == 2 * NG - 1))

                # 1/den = Exp(-Ln(den + 1e-20)); unused rows are exact 0,
                # Ln(1e-20) = -46 -> Exp(46) finite, killed by 0 selector.
                dinv = pden.tile([128, tc], bf16, tag="dinv")
                lnd = pact.tile([128, tc], f32, tag="lnd")
                nc.scalar.activation(lnd, dp, Ln, bias=epsb)
                nc.scalar.activation(dinv, lnd, Exp, scale=-1.0)

                def bcast_pe(s, j, tag):
                    """den_inv rows {32j+2s, 32j+2s+1} -> [128, tc] via
                    selector matmul + ACT drain."""
                    bc_ps = ps_bc.tile([128, tc], f32, tag="bc")
                    nc.tensor.matmul(bc_ps, esel_sb[:, 4 * s + j, :], dinv,
                                     start=True, stop=True)
                    bc = pbc.tile([128, tc], bf16, tag=tag)
                    nc.scalar.copy(bc, bc_ps)
                    return bc

                for j in range(NG):
                    bcf = bcast_pe(0, j, "bcf")
                    yf = py.tile([128, tc], bf16, tag="yf")
                    nc.vector.tensor_mul(yf, numf_j[j], bcf)
                    yf_j[j] = yf

                    bcs = bcast_pe(1, j, "bcs")
                    ys = py.tile([128, tc], bf16, tag="ys")
                    nc.vector.tensor_mul(ys, nums_j[j], bcs)
                    ys_j[j] = ys

                ystash[c] = yf_j + ys_j

            def tail_b_slice(c, ts_):
                """out-projection for t-subtile ts_ of chunk c."""
                t0 = c * tc
                ytiles = ystash[c]
                osb = pout.tile([128, 1024], bf16, tag="osb")
                for ob in range(2):
                    pso = ps_out.tile([128, 512], f32, tag="pso")
                    for yt in range(8):
                        nc.tensor.matmul(
                            pso,
                            ytiles[yt][:, 128 * ts_:128 * ts_ + 128],
                            wo_sb[:, yt, 512 * ob:512 * ob + 512],
                            start=(yt == 0), stop=(yt == 7))
                    nc.scalar.copy(osb[:, 512 * ob:512 * ob + 512], pso)
                nc.sync.dma_start(
                    out=yout[t0 + 128 * ts_:t0 + 128 * ts_ + 128, :],
                    in_=osb)
                if ts_ == tc // 128 - 1:
                    del ystash[c]

            ystash = {}
            # 2-deep software pipeline, group-interleaved:
            #   front(c) groups  ||  outproj slices of chunk c-2  ||  tailA(c-1)
            nslice = tc // 128
            load_wq(0)
            xcs = {0: load_xc(0)}
            load_dec()
            for j in range(1, NG):
                load_wq(j)
            for c in range(chunks):
                xc = xcs.pop(c)
                for j in range(NG):
                    front_group(c, j, xc)
                    if c >= 2:
                        for k in range(nslice * j // NG, nslice * (j + 1) // NG):
                            tail_b_slice(c - 2, k)
                    if j == 1 and c >= 1:
                        stage_tail_a(c - 1)
                if c == 0:
                    load_late_consts()
                if c + 1 < chunks:
                    xcs[c + 1] = load_xc(c + 1)
            stage_tail_a(chunks - 1)
            for cc in (chunks - 2, chunks - 1):
                if cc >= 0 and cc in ystash:
                    for k in range(nslice):
                        tail_b_slice(cc, k)

            # --- final states: last column of each scan ---
            fin = const.tile([128, 16], bf16)
            for s, tag in enumerate(("rkvf", "rksf", "rkvs", "rkss")):
                for j in range(NG):
                    nc.vector.tensor_copy(fin[:, 4 * s + j:4 * s + j + 1],
                                          prev[tag][j][:, tc - 1:tc])
            nc.sync.dma_start(out=finals, in_=fin)

    nc.compile()
    return nc


def _host_inputs(x, w_qkv, w_out, alpha, beta, t=T, tc=TC):
    """Build the 8 per-core input maps (host-side shard + transpose + cast)."""
    x = np.asarray(x, dtype=np.float32)
    w_qkv = np.asarray(w_qkv, dtype=np.float32)
    w_out = np.asarray(w_out, dtype=np.float32)
    alpha = np.asarray(alpha, dtype=np.float32)
    beta = np.asarray(beta, dtype=np.float32)

    d_a = 1.0 / (1.0 + np.exp(-alpha.astype(np.float64)))
    d_b = 1.0 / (1.0 + np.exp(-beta.astype(np.float64)))
    d_a = d_a.astype(np.float32)
    d_b = d_b.astype(np.float32)

    esel = np.zeros((2 * NG, 128, 128), dtype=BF16)
    ehead = np.zeros((2 * NG, 128, 128), dtype=BF16)
    for s in range(2):
        for j in range(NG):
            r = 32 * j + 2 * s
            esel[4 * s + j, r, 0:64] = 1
            esel[4 * s + j, r + 1, 64:128] = 1
            ehead[4 * s + j, 0:64, r] = 1
            ehead[4 * s + j, 64:128, r + 1] = 1

    in_maps = []
    for core in range(NCORES):
        b, g = core // 2, core % 2
        xt = np.ascontiguousarray(x[b, :t].T).reshape(8, 128, t).astype(BF16)
        blocks = []
        for j in range(NG):
            for off in (0, 1024, 2048):
                c0 = off + 512 * g + 128 * j
                blocks.append(w_qkv[:, c0:c0 + 128])
        wqc = np.concatenate(blocks, axis=1).reshape(8, 128, 1536)
        wqc = np.ascontiguousarray(wqc).astype(BF16)
        fo = w_out[512 * g:512 * g + 512, :]
        so = w_out[1024 + 512 * g:1024 + 512 * g + 512, :]
        woc = np.concatenate([fo, so], axis=0).reshape(8, 128, 1024)
        woc = np.ascontiguousarray(woc).astype(BF16)

        deca = np.zeros((NG, 128, tc), dtype=np.float32)
        decb = np.zeros((NG, 128, tc), dtype=np.float32)
        for j in range(NG):
            deca[j, 0:64, :] = d_a[8 * g + 2 * j]
            deca[j, 64:128, :] = d_a[8 * g + 2 * j + 1]
            decb[j, 0:64, :] = d_b[8 * g + 2 * j]
            decb[j, 64:128, :] = d_b[8 * g + 2 * j + 1]

        in_maps.append({
            "xT": xt, "wq": wqc, "wo": woc,
            "deca": deca, "decb": decb,
            "esel": esel, "ehead": ehead,
        })
    return in_maps


def _assemble(results, t=T):
    out = np.zeros((B, t, HID), dtype=np.float32)
    kv_f1 = np.zeros((B, H, D), dtype=np.float32)
    ks_f1 = np.zeros((B, H, D), dtype=np.float32)
    kv_s1 = np.zeros((B, H, D), dtype=np.float32)
    ks_s1 = np.zeros((B, H, D), dtype=np.float32)
    for core in range(NCORES):
        b, g = core // 2, core % 2
        out[b] += results[core]["yout"].astype(np.float32)
        fin = results[core]["finals"].astype(np.float32)
        for s, arr in enumerate((kv_f1, ks_f1, kv_s1, ks_s1)):
            for j in range(NG):
                col = fin[:, 4 * s + j]
                arr[b, 8 * g + 2 * j, :] = col[0:64]
                arr[b, 8 * g + 2 * j + 1, :] = col[64:128]
    return out, (kv_f1, ks_f1, kv_s1, ks_s1)


def kernel(x, w_qkv, w_out, alpha, beta, _trace=False):
    key = (T, TC)
    if key not in _BUILD_CACHE:
        _BUILD_CACHE[key] = build(T, TC)
    nc = _BUILD_CACHE[key]
    in_maps = _host_inputs(x, w_qkv, w_out, alpha, beta, T, TC)
    kw = {}
    if _trace:
        kw["trace"] = True
    res = run_bass_kernel_spmd(nc, in_maps, list(range(NCORES)), **kw)
    outs = _assemble(res.results, T)
    if _trace:
        kernel.last_exec_time_ns = res.exec_time_ns
        kernel.last_result = res
    return outs


# revision 56
# speedup vs baseline: 1.3845x; 1.0163x over previous
"""Dual-state linear attention Trainium2 kernel (8 NeuronCores, SPMD).

Sharding: core = (batch b, head-group g): b = core // 2, g = core % 2.
Each core processes batch b and heads 8g..8g+7 (feature-sharded w_qkv /
w_out slices).  The out-projection partial sums of the two head groups of
each batch are added on the host.

On-chip layout: feature-on-partition, time-on-free ("transposed"); x is
pre-transposed and bf16-cast on the host so no on-device transposes exist.
 - qkv matmul: psum[j 128, t TC] = sum_a wq[k-tile, j-tile].T @ xT[k-tile, chunk]
 - phi(x) = elu(x)+1 = relu(x) + exp(min(x, 0)):
     rneg = Relu(-x) (ACT), e = Exp(-rneg) (ACT),
     phi  = (x max 0) + e   (DVE scalar_tensor_tensor, drains PSUM)
 - decay scans: DVE tensor_tensor_scan along the free (time) dim, fp32
     decay tiles (bf16 decay would distort 1/(1-d) by ~6%), chained
     across chunks via initial = prev[:, -1:]
 - den[h,t] = sum_d q*run_ks: selector matmuls accumulate both decays'
     head-sums into ONE psum bank at rows 32j + 2s + e
 - 1/den = Exp(-Ln(den + 1e-20)) on ACT (Reciprocal is blocked; unused
     rows are exact 0 -> finite garbage killed by 0 selector weights)
 - broadcast 1/den rows to 64-row blocks: selector matmul + ACT drain
 - Y = (q * run_kv) * den_inv_bcast (DVE, bf16 2x mode)
 - out[t, o] = sum_y Y[y-tile, t-sub].T @ wo[y-tile, o-bank] (PE),
     ACT-drained to bf16, DMA out; host sums the 2 head-group partials.

Scheduling: 2-deep software pipeline, interleaved at feature-group
granularity so the in-order PE queue can fill qkv stalls with the
out-projection of chunk c-2 (whose Y tiles are certainly ready):
  front(c) group j  ||  outproj slice j of chunk c-2, then tailA(c-1).
Measured ~376 us on silicon (PE busy ~323 us, DVE ~283 us of which the
four scans are ~151 us at the hardware's 2.09 cycles/element).
"""
import sys

sys.path.insert(0, "/opt/trn_rl_repo")

import numpy as np
import ml_dtypes

import concourse.bass as bass
import concourse.bacc as bacc
import concourse.tile as tile
from concourse import mybir
from concourse.bass_utils import run_bass_kernel_spmd

BF16 = ml_dtypes.bfloat16

B, T, HID, H, D = 4, 4096, 1024, 16, 64
NCORES = 8
TC = 512            # time-chunk
NG = 4              # feature partition-groups per core (8 heads x 64 = 512 rows)

_BUILD_CACHE = {}


def build(t=T, tc=TC):
    """Build the SPMD Bass program. Returns (nc, names)."""
    chunks = t // tc
    nc = bacc.Bacc("TRN2", target_bir_lowering=False, debug=False,
                   enable_asserts=False, num_devices=NCORES)
    f32, bf16 = mybir.dt.float32, mybir.dt.bfloat16

    xT = nc.dram_tensor("xT", [8, 128, t], bf16, kind="ExternalInput").ap()
    wq = nc.dram_tensor("wq", [8, 128, 1536], bf16, kind="ExternalInput").ap()
    wo = nc.dram_tensor("wo", [8, 128, 1024], bf16, kind="ExternalInput").ap()
    deca = nc.dram_tensor("deca", [NG, 128, tc], f32, kind="ExternalInput").ap()
    decb = nc.dram_tensor("decb", [NG, 128, tc], f32, kind="ExternalInput").ap()
    esel = nc.dram_tensor("esel", [2 * NG, 128, 128], bf16, kind="ExternalInput").ap()
    ehead = nc.dram_tensor("ehead", [2 * NG, 128, 128], bf16, kind="ExternalInput").ap()
    yout = nc.dram_tensor("yout", [t, 1024], bf16, kind="ExternalOutput").ap()
    finals = nc.dram_tensor("finals", [128, 16], bf16, kind="ExternalOutput").ap()

    Relu = mybir.ActivationFunctionType.Relu
    Exp = mybir.ActivationFunctionType.Exp
    Ln = mybir.ActivationFunctionType.Ln
    MUL = mybir.AluOpType.mult
    ADD = mybir.AluOpType.add
    MAX = mybir.AluOpType.max

    with tile.TileContext(nc) as tc_:
        import contextlib
        ctx = contextlib.ExitStack()
        with ctx:
            const = ctx.enter_context(tc_.tile_pool(name="const", bufs=1))
            px = ctx.enter_context(tc_.tile_pool(name="px", bufs=2))
            pact = ctx.enter_context(tc_.tile_pool(name="pact", bufs=2))
            pqk = ctx.enter_context(tc_.tile_pool(name="pqk", bufs=5))
            pscan = ctx.enter_context(tc_.tile_pool(name="pscan", bufs=8))
            pmid = ctx.enter_context(tc_.tile_pool(name="pmid", bufs=8))
            pbc = ctx.enter_context(tc_.tile_pool(name="pbc", bufs=3))
            pv = ctx.enter_context(tc_.tile_pool(name="pv", bufs=2))
            py = ctx.enter_context(tc_.tile_pool(name="py", bufs=12))
            pden = ctx.enter_context(tc_.tile_pool(name="pden", bufs=2))
            pout = ctx.enter_context(tc_.tile_pool(name="pout", bufs=2))
            ps_q = ctx.enter_context(
                tc_.tile_pool(name="ps_q", bufs=2, space="PSUM"))
            ps_k = ctx.enter_context(
                tc_.tile_pool(name="ps_k", bufs=1, space="PSUM"))
            ps_v = ctx.enter_context(
                tc_.tile_pool(name="ps_v", bufs=1, space="PSUM"))
            ps_bc = ctx.enter_context(
                tc_.tile_pool(name="ps_bc", bufs=2, space="PSUM"))
            ps_out = ctx.enter_context(
                tc_.tile_pool(name="ps_out", bufs=2, space="PSUM"))

            # constants / weights resident in SBUF (emitted inside the
            # pipeline driver: xc(0) and group-0 blocks first)
            wq_sb = const.tile([128, 8, 12, 128], bf16)
            wqr = wq.rearrange("a p (blk n) -> a p blk n", n=128)

            def load_wq(j):
                for a in range(8):
                    nc.sync.dma_start(out=wq_sb[:, a, 3 * j:3 * j + 3, :],
                                      in_=wqr[a, :, 3 * j:3 * j + 3, :])
            deca_sb = const.tile([128, NG, tc], f32)
            decb_sb = const.tile([128, NG, tc], f32)

            def load_dec():
                for j in range(NG):
                    nc.sync.dma_start(out=deca_sb[:, j, :], in_=deca[j])
                    nc.sync.dma_start(out=decb_sb[:, j, :], in_=decb[j])
            ehead_sb = const.tile([128, 2 * NG, 128], bf16)
            esel_sb = const.tile([128, 2 * NG, 128], bf16)
            wo_sb = const.tile([128, 8, 1024], bf16)
            epsb = const.tile([128, 1], f32)
            nc.vector.memset(epsb, 1e-20)

            def load_late_consts():
                for j in range(2 * NG):
                    nc.sync.dma_start(out=ehead_sb[:, j, :], in_=ehead[j])
                    nc.sync.dma_start(out=esel_sb[:, j, :], in_=esel[j])
                for a in range(8):
                    nc.sync.dma_start(out=wo_sb[:, a, :], in_=wo[a])

            prev = {tag: [None] * NG
                    for tag in ("rkvf", "rkvs", "rksf", "rkss")}
            stash = {}   # chunk -> (tmpf_j, tmps_j, numf_j, nums_j)

            def load_xc(c):
                t0 = c * tc
                xc = px.tile([128, 8, tc], bf16, tag="xc")
                for a in range(8):
                    nc.sync.dma_start(out=xc[:, a, :], in_=xT[a, :, t0:t0 + tc])
                return xc

            def front_group(c, j, xc):
                """qkv matmuls + phi + kv + scans + tmp/num for (chunk, grp)."""
                if j == 0:
                    stash[c] = ([None] * NG, [None] * NG,
                                [None] * NG, [None] * NG)
                tmpf_j, tmps_j, numf_j, nums_j = stash[c]
                if True:
                    psq = ps_q.tile([128, tc], f32, tag="psq")
                    psk = ps_k.tile([128, tc], f32, tag="psk")
                    psv = ps_v.tile([128, tc], f32, tag="psv")
                    for qi, ps in enumerate((psq, psk, psv)):
                        for a in range(8):
                            nc.tensor.matmul(
                                ps, wq_sb[:, a, 3 * j + qi, :],
                                xc[:, a, :], start=(a == 0), stop=(a == 7))

                    qsb = pqk.tile([128, tc], bf16, tag="q")
                    ksb = pqk.tile([128, tc], bf16, tag="k")
                    for src, dst in ((psq, qsb), (psk, ksb)):
                        rneg = pact.tile([128, tc], f32, tag="rneg")
                        nc.scalar.activation(rneg, src, Relu, scale=-1.0)
                        ex = pact.tile([128, tc], f32, tag="ex")
                        nc.scalar.activation(ex, rneg, Exp, scale=-1.0)
                        nc.vector.scalar_tensor_tensor(
                            dst, src, 0.0, ex, MAX, ADD)

                    vsb = pv.tile([128, tc], bf16, tag="v")
                    nc.scalar.copy(vsb, psv)
                    kvt = pqk.tile([128, tc], bf16, tag="kv")
                    nc.vector.tensor_mul(kvt, ksb, vsb)

                    souts = {}
                    for tag, src, dsb in (
                            ("rkvf", kvt, deca_sb), ("rkvs", kvt, decb_sb),
                            ("rksf", ksb, deca_sb), ("rkss", ksb, decb_sb)):
                        so = pscan.tile([128, tc], bf16, tag=tag)
                        init = (0.0 if c == 0
                                else prev[tag][j][:, tc - 1:tc])
                        nc.vector.tensor_tensor_scan(
                            so, dsb[:, j, :], src, init, MUL, ADD)
                        prev[tag][j] = so
                        souts[tag] = so

                    tmpf = pmid.tile([128, tc], bf16, tag="tmpf")
                    nc.vector.tensor_mul(tmpf, qsb, souts["rksf"])
                    tmps = pmid.tile([128, tc], bf16, tag="tmps")
                    nc.vector.tensor_mul(tmps, qsb, souts["rkss"])
                    numf = pmid.tile([128, tc], bf16, tag="numf")
                    nc.vector.tensor_mul(numf, qsb, souts["rkvf"])
                    nums = pmid.tile([128, tc], bf16, tag="nums")
                    nc.vector.tensor_mul(nums, qsb, souts["rkvs"])
                    tmpf_j[j], tmps_j[j] = tmpf, tmps
                    numf_j[j], nums_j[j] = numf, nums

            def stage_tail_a(c, mid_cb=None):
                """den + 1/den + broadcast + Y for chunk c."""
                tmpf_j, tmps_j, numf_j, nums_j = stash.pop(c)
                yf_j, ys_j = [None] * NG, [None] * NG

                # both decays' denominators in ONE psum bank:
                # group j / decay s / head-parity e at row 32j + 2s + e
                dp = ps_bc.tile([128, tc], f32, tag="bc")
                for i, tm in enumerate(tmpf_j + tmps_j):
                    s, j = divmod(i, NG)
                    nc.tensor.matmul(dp, ehead_sb[:, 4 * s + j, :], tm,
                                     start=(i == 0), stop=(i == 2 * NG - 1))

                # 1/den = Exp(-Ln(den + 1e-20)); unused rows are exact 0,
                # Ln(1e-20) = -46 -> Exp(46) finite, killed by 0 selector.
                dinv = pden.tile([128, tc], bf16, tag="dinv")
                lnd = pact.tile([128, tc], f32, tag="lnd")
                nc.scalar.activation(lnd, dp, Ln, bias=epsb)
                nc.scalar.activation(dinv, lnd, Exp, scale=-1.0)
                if mid_cb is not None:
                    mid_cb()

                def bcast_pe(s, j, tag):
                    """den_inv rows {32j+2s, 32j+2s+1} -> [128, tc] via
                    selector matmul + ACT drain."""
                    bc_ps = ps_bc.tile([128, tc], f32, tag="bc")
                    nc.tensor.matmul(bc_ps, esel_sb[:, 4 * s + j, :], dinv,
                                     start=True, stop=True)
                    bc = pbc.tile([128, tc], bf16, tag=tag)
                    nc.scalar.copy(bc, bc_ps)
                    return bc

                for j in range(NG):
                    bcf = bcast_pe(0, j, "bcf")
                    yf = py.tile([128, tc], bf16, tag="yf")
                    nc.vector.tensor_mul(yf, numf_j[j], bcf)
                    yf_j[j] = yf

                    bcs = bcast_pe(1, j, "bcs")
                    ys = py.tile([128, tc], bf16, tag="ys")
                    nc.vector.tensor_mul(ys, nums_j[j], bcs)
                    ys_j[j] = ys

                ystash[c] = yf_j + ys_j

            def tail_b_slice(c, ts_):
                """out-projection for t-subtile ts_ of chunk c."""
                t0 = c * tc
                ytiles = ystash[c]
                osb = pout.tile([128, 1024], bf16, tag="osb")
                for ob in range(2):
                    pso = ps_out.tile([128, 512], f32, tag="pso")
                    for yt in range(8):
                        nc.tensor.matmul(
                            pso,
                            ytiles[yt][:, 128 * ts_:128 * ts_ + 128],
                            wo_sb[:, yt, 512 * ob:512 * ob + 512],
                            start=(yt == 0), stop=(yt == 7))
                    nc.scalar.copy(osb[:, 512 * ob:512 * ob + 512], pso)
                nc.sync.dma_start(
                    out=yout[t0 + 128 * ts_:t0 + 128 * ts_ + 128, :],
                    in_=osb)
                if ts_ == tc // 128 - 1:
                    del ystash[c]

            ystash = {}
            # 2-deep software pipeline, group-interleaved:
            #   front(c) groups  ||  outproj slices of chunk c-2  ||  tailA(c-1)
            nslice = tc // 128
            load_wq(0)
            xcs = {0: load_xc(0)}
            load_dec()
            for j in range(1, NG):
                load_wq(j)
            for c in range(chunks):
                xc = xcs.pop(c)
                def emit_fg(j):
                    front_group(c, j, xc)
                    if c >= 2:
                        for k in range(nslice * j // NG, nslice * (j + 1) // NG):
                            tail_b_slice(c - 2, k)

                for j in range(NG):
                    if j == 3 and c >= 1:
                        continue  # emitted inside tailA's mid_cb below
                    emit_fg(j)
                    if j == 2 and c >= 1:
                        stage_tail_a(c - 1, mid_cb=lambda: emit_fg(3))
                if c == 0:
                    load_late_consts()
                if c + 1 < chunks:
                    xcs[c + 1] = load_xc(c + 1)
            stage_tail_a(
                chunks - 1,
                mid_cb=lambda: [tail_b_slice(chunks - 2, k)
                                for k in range(nslice)
                                if chunks - 2 in ystash])
            for cc in (chunks - 2, chunks - 1):
                if cc >= 0 and cc in ystash:
                    for k in range(nslice):
                        tail_b_slice(cc, k)

            # --- final states: last column of each scan ---
            fin = const.tile([128, 16], bf16)
            for s, tag in enumerate(("rkvf", "rksf", "rkvs", "rkss")):
                for j in range(NG):
                    nc.vector.tensor_copy(fin[:, 4 * s + j:4 * s + j + 1],
                                          prev[tag][j][:, tc - 1:tc])
            nc.sync.dma_start(out=finals, in_=fin)

    nc.compile()
    return nc


def _host_inputs(x, w_qkv, w_out, alpha, beta, t=T, tc=TC):
    """Build the 8 per-core input maps (host-side shard + transpose + cast)."""
    x = np.asarray(x, dtype=np.float32)
    w_qkv = np.asarray(w_qkv, dtype=np.float32)
    w_out = np.asarray(w_out, dtype=np.float32)
    alpha = np.asarray(alpha, dtype=np.float32)
    beta = np.asarray(beta, dtype=np.float32)

    d_a = 1.0 / (1.0 + np.exp(-alpha.astype(np.float64)))
    d_b = 1.0 / (1.0 + np.exp(-beta.astype(np.float64)))
    d_a = d_a.astype(np.float32)
    d_b = d_b.astype(np.float32)

    esel = np.zeros((2 * NG, 128, 128), dtype=BF16)
    ehead = np.zeros((2 * NG, 128, 128), dtype=BF16)
    for s in range(2):
        for j in range(NG):
            r = 32 * j + 2 * s
            esel[4 * s + j, r, 0:64] = 1
            esel[4 * s + j, r + 1, 64:128] = 1
            ehead[4 * s + j, 0:64, r] = 1
            ehead[4 * s + j, 64:128, r + 1] = 1

    in_maps = []
    for core in range(NCORES):
        b, g = core // 2, core % 2
        xt = np.ascontiguousarray(x[b, :t].T).reshape(8, 128, t).astype(BF16)
        blocks = []
        for j in range(NG):
            for off in (0, 1024, 2048):
                c0 = off + 512 * g + 128 * j
                blocks.append(w_qkv[:, c0:c0 + 128])
        wqc = np.concatenate(blocks, axis=1).reshape(8, 128, 1536)
        wqc = np.ascontiguousarray(wqc).astype(BF16)
        fo = w_out[512 * g:512 * g + 512, :]
        so = w_out[1024 + 512 * g:1024 + 512 * g + 512, :]
        woc = np.concatenate([fo, so], axis=0).reshape(8, 128, 1024)
        woc = np.ascontiguousarray(woc).astype(BF16)

        deca = np.zeros((NG, 128, tc), dtype=np.float32)
        decb = np.zeros((NG, 128, tc), dtype=np.float32)
        for j in range(NG):
            deca[j, 0:64, :] = d_a[8 * g + 2 * j]
            deca[j, 64:128, :] = d_a[8 * g + 2 * j + 1]
            decb[j, 0:64, :] = d_b[8 * g + 2 * j]
            decb[j, 64:128, :] = d_b[8 * g + 2 * j + 1]

        in_maps.append({
            "xT": xt, "wq": wqc, "wo": woc,
            "deca": deca, "decb": decb,
            "esel": esel, "ehead": ehead,
        })
    return in_maps


def _assemble(results, t=T):
    out = np.zeros((B, t, HID), dtype=np.float32)
    kv_f1 = np.zeros((B, H, D), dtype=np.float32)
    ks_f1 = np.zeros((B, H, D), dtype=np.float32)
    kv_s1 = np.zeros((B, H, D), dtype=np.float32)
    ks_s1 = np.zeros((B, H, D), dtype=np.float32)
    for core in range(NCORES):
        b, g = core // 2, core % 2
        out[b] += results[core]["yout"].astype(np.float32)
        fin = results[core]["finals"].astype(np.float32)
        for s, arr in enumerate((kv_f1, ks_f1, kv_s1, ks_s1)):
            for j in range(NG):
                col = fin[:, 4 * s + j]
                arr[b, 8 * g + 2 * j, :] = col[0:64]
                arr[b, 8 * g + 2 * j + 1, :] = col[64:128]
    return out, (kv_f1, ks_f1, kv_s1, ks_s1)


def kernel(x, w_qkv, w_out, alpha, beta, _trace=False):
    key = (T, TC)
    if key not in _BUILD_CACHE:
        _BUILD_CACHE[key] = build(T, TC)
    nc = _BUILD_CACHE[key]
    in_maps = _host_inputs(x, w_qkv, w_out, alpha, beta, T, TC)
    kw = {}
    if _trace:
        kw["trace"] = True
    res = run_bass_kernel_spmd(nc, in_maps, list(range(NCORES)), **kw)
    outs = _assemble(res.results, T)
    if _trace:
        kernel.last_exec_time_ns = res.exec_time_ns
        kernel.last_result = res
    return outs
